# revision 10
# baseline (speedup 1.0000x reference)
"""CRF-RNN (dense Gaussian CRF mean-field) Trainium2 kernel, 8 NeuronCores.

Strategy (v2 — factorized bilateral)
------------------------------------
N = 8*32*32 = 8192 voxels, L = 21 labels, 5 mean-field iterations.
- The bilateral kernel K_b is rank-36: K_b[i,c] = R(i).C(c) with the
  quadratic exp fit PC*(PA - d^2/2)^2 (max rel err 1.3e-3, largely
  cancelled by the host-computed normalizer built from the same fit).
  So q@K_b = (q@R^T)@C where P = q@R^T is a tiny [36,21] matrix —
  K_b is NEVER materialized. Each core computes its own-slice partial
  P_r (8 small matmuls) and ships it inside the per-iteration AllGather;
  the full bilateral message is then 2 matmuls P^T @ featc.
- Spatial kernel is separable: z-mix (DVE weighted sums over the 8
  gathered slices) then one pass of brt @ (Gy x Gx) [1024x1024, fp16].
  The own-slice term is prestarted during the AllGather (zcoo scale).
- Column sharding: core r owns z-slice r. One fp16 AllGather per
  iteration carries [q_own (168 cols) | P_partial (21 cols)].
- Unary is folded into the label-mixing matmul: cat rows 42:63 hold
  unown^T and w12t rows 42:63 are the identity, so ps_ct = messages +
  unary comes out of the PE directly and softmax reads PSUM.
- 1/norm_b is folded into featc (x1024 to stay in fp16 normal range,
  /1024 in the bilateral rows of w12t); 1/norm_s into kyx + z coefs.
- Per-core unary is block-permuted own-slice-first so the SPMD program
  is core-independent (zcor0 input carries the matching z-coefs).
- Iteration-0 P is gathered via an early tiny AllGather (CC0) that
  overlaps the kyx DMA and the prestarted own-slice spatial matmuls.
"""

import numpy as np

ALPHA, BETA, GAMMA = 160.0, 3.0, 3.0
NUM_ITER = 5
L, D, H, W = 21, 8, 32, 32
NC = 8
NYX = H * W            # 1024
N = D * NYX            # 8192
NT = NYX // 128        # 8 chunks per slice
FB = NT * L            # 168  free width of one q block
NF = 36                # polynomial feature-map width
FSCALE = 1024.0        # featc pre-scale (folded back via w12t bil rows)
CCW = FB + 24          # 192: 168 q cols + 21 P cols + 3 pad
CC0W = 32              # CC0 payload width (P partial only)
CATP = 85              # cat rows: 0:21 bil, 32:53 spat, 64:85 unownT
                       # (engine partition bases must be 32-aligned)

PA = 2.105             # quadratic exp fit: exp(x) ~= PC*(x+PA)^2 on [-0.21,0]
PC = 0.22538087

_CACHE = {}


def _build_nc():
    import concourse.bass as bass  # noqa: F401
    import concourse.bacc as bacc
    import concourse.mybir as mybir
    import concourse.tile as tile
    import concourse.tile_utils as tile_utils

    try:
        tile_utils.max_sbuf_usage = 204 * 1024
    except Exception:
        pass

    f32 = mybir.dt.float32
    f16 = mybir.dt.float16
    AF = mybir.ActivationFunctionType
    OP = mybir.AluOpType

    nc = bacc.Bacc(None, target_bir_lowering=False, num_devices=NC)

    unary_d = nc.declare_dram_parameter("unaryt", [128, NC * FB], f16, isOutput=False)
    featrt_d = nc.declare_dram_parameter("featrt", [128, NT * NF], f16, isOutput=False)
    featc_d = nc.declare_dram_parameter("featc", [NF, NYX], f16, isOutput=False)
    kyx_d = nc.declare_dram_parameter("kyx", [128, NT * NYX], f16, isOutput=False)
    zcor_d = nc.declare_dram_parameter("zcor", [128, NC], f32, isOutput=False)
    zcor0_d = nc.declare_dram_parameter("zcor0", [128, NC], f32, isOutput=False)
    zcoo_d = nc.declare_dram_parameter("zcoo", [128, 1], f32, isOutput=False)
    w12t_d = nc.declare_dram_parameter("w12t", [CATP, L], f16, isOutput=False)
    unownt_d = nc.declare_dram_parameter("unownt", [L, NYX], f16, isOutput=False)
    out_d = nc.declare_dram_parameter("out", [128, FB], f32, isOutput=True)

    with tile.TileContext(nc) as tc:
        with (
            tc.tile_pool(name="persist", bufs=1) as pp,
            tc.tile_pool(name="work", bufs=4) as wp,
            tc.tile_pool(name="slots", bufs=2) as slp,
            tc.tile_pool(name="stag", bufs=2) as stp,
            tc.tile_pool(name="dram", bufs=2, space="DRAM") as dp,
            tc.tile_pool(name="ps_sp", bufs=2, space="PSUM") as ps_sp,
            tc.tile_pool(name="ps_bil", bufs=1, space="PSUM") as ps_bil_p,
            tc.tile_pool(name="ps_ct", bufs=1, space="PSUM") as ps_ct_p,
            tc.tile_pool(name="ps_p", bufs=1, space="PSUM") as ps_p,
        ):
            sb_unary = pp.tile([128, NC * FB], f16, tag="unary")
            sb_featrt = pp.tile([128, NT * NF], f16, tag="featrt")
            sb_featc = pp.tile([NF, NYX], f16, tag="featc")
            sb_kyx = pp.tile([128, NT * NYX], f16, tag="kyx")
            sb_zcor = pp.tile([128, NC], f32, tag="zcor")
            sb_zcor0 = pp.tile([128, NC], f32, tag="zcor0")
            sb_zcoo = pp.tile([128, 1], f32, tag="zcoo")
            sb_w12t = pp.tile([CATP, L], f16, tag="w12t")
            cats = [pp.tile([CATP, 512], f16, tag=f"cat{h}", name=f"cat{h}")
                    for h in range(2)]
            sb_ex0 = pp.tile([128, NC * FB], f32, tag="ex0")
            sb_red0 = pp.tile([128, NC * NT], f32, tag="red0")
            sb_rcp0 = pp.tile([128, NC * NT], f32, tag="rcp0")
            sb_q0 = pp.tile([128, NC * FB], f16, tag="q0")
            sb_out = pp.tile([128, FB], f32, tag="outt")
            sb_warm = pp.tile([128, 1], f32, tag="warm")
            sb_warm2 = pp.tile([128, 1], f32, tag="warm2")
            sb_wu = pp.tile([128, 8], f16, tag="wu")

            # input DMAs: unary gates softmax-0, featrt gates P0; kyx (2MB)
            # streams behind them and only gates the prestarted spatial.
            nc.sync.dma_start(sb_unary[:, :], unary_d[:, :])
            nc.sync.dma_start(sb_featrt[:, :], featrt_d[:, :])
            nc.sync.dma_start(sb_featc[:, :], featc_d[:, :])
            nc.sync.dma_start(sb_zcor[:, :], zcor_d[:, :])
            nc.sync.dma_start(sb_zcor0[:, :], zcor0_d[:, :])
            nc.sync.dma_start(sb_zcoo[:, :], zcoo_d[:, :])
            nc.sync.dma_start(sb_w12t[:, :], w12t_d[:, :])
            sb_unt = pp.tile([L, NYX], f16, tag="unt")
            nc.sync.dma_start(sb_unt[:, :], unownt_d[:, :])
            nc.sync.dma_start(sb_kyx[:, :], kyx_d[:, :])
            # gap rows meet w12t zero rows in the PE; memset the whole cat
            # once (base-0 partition access) so uninitialized-SBUF NaN
            # patterns can't poison the MACs, then land unownT at rows 64+.
            for h in range(2):
                nc.vector.memset(cats[h][:, :], 0.0)
            nc.vector.tensor_copy(cats[0][64:CATP, :], sb_unt[:, 0:512])
            nc.scalar.copy(cats[1][64:CATP, :], sb_unt[:, 512:NYX])

            # exp table prewarm (2.7us, overlaps DMA) + collective warmup
            nc.vector.memset(sb_warm[:, :], 0.0)
            nc.scalar.activation(sb_warm2[:, :], sb_warm[:, :], AF.Exp)
            nc.vector.memset(sb_wu[:, :], 0.0)
            wu_in = dp.tile([128, 8], f16, tag="wuin")
            wu_out = dp.tile([128 * NC, 8], f16, tag="wuout")
            nc.sync.dma_start(wu_in[:, :], sb_wu[:, :])
            nc.gpsimd.collective_compute(
                "AllGather", mybir.AluOpType.bypass,
                replica_groups=[list(range(NC))],
                ins=[wu_in.opt()], outs=[wu_out.opt()],
            )

            kyx_v = sb_kyx[:, :].rearrange("p (k c) -> p k c", c=NYX)

            # ---------------- iter-0 softmax over full permuted unary ----
            ex0_v = sb_ex0[:, :].rearrange("p (g l) -> p g l", l=L)
            q0_v = sb_q0[:, :].rearrange("p (g l) -> p g l", l=L)
            HW2 = NC * FB // 2
            G2 = NC * NT // 2
            for h in range(2):
                cs = slice(h * HW2, (h + 1) * HW2)
                gs = slice(h * G2, (h + 1) * G2)
                nc.scalar.activation(sb_ex0[:, cs], sb_unary[:, cs], AF.Exp)
                nc.vector.tensor_reduce(
                    sb_red0[:, gs], ex0_v[:, gs, :], mybir.AxisListType.X, OP.add)
                nc.vector.reciprocal(sb_rcp0[:, gs], sb_red0[:, gs])
                nc.vector.tensor_tensor(
                    q0_v[:, gs, :], ex0_v[:, gs, :],
                    sb_rcp0[:, gs].broadcast_to([128, G2, L]), OP.mult)

            # ---------------- CC0: gather iter-0 P partials ----------------
            psP0 = ps_p.tile([NF, L], f32, tag="pp")
            for t in range(NT):
                nc.tensor.matmul(
                    psP0[:, :], sb_featrt[:, t * NF:(t + 1) * NF],
                    sb_q0[:, t * L:(t + 1) * L],
                    start=(t == 0), stop=(t == NT - 1))
            stag0 = stp.tile([NF, CC0W], f16, tag="stag0")
            nc.scalar.copy(stag0[:, 0:L], psP0[:, :])
            cc0_in = dp.tile([NF, CC0W], f16, tag="cc0in")
            cc0_out = dp.tile([NF * NC, CC0W], f16, tag="cc0out")
            nc.sync.dma_start(cc0_in[:, :], stag0[:, :])
            nc.gpsimd.collective_compute(
                "AllGather", mybir.AluOpType.bypass,
                replica_groups=[list(range(NC))],
                ins=[cc0_in.opt()], outs=[cc0_out.opt()],
            )
            slots0 = slp.tile([NF, NC * CC0W], f16, tag="slots0")
            nc.sync.dma_start(
                slots0[:, :].rearrange("p (d f) -> p d f", d=NC),
                cc0_out[:, :].rearrange("(d p) f -> p d f", p=NF))

            # prestart iter-0 own-slice spatial (overlaps CC0 + kyx DMA)
            bo0 = wp.tile([128, FB], f16, tag="bo")
            nc.vector.tensor_scalar_mul(bo0[:, :], sb_q0[:, 0:FB], sb_zcoo[:, 0:1])
            bo0_v = bo0[:, :].rearrange("p (t l) -> p t l", l=L)
            sp_cur = ps_sp.tile([L, NYX], f32, tag="spat")
            for k in range(NT):
                for hb in range(2):
                    nc.tensor.matmul(
                        sp_cur[:, hb * 512:(hb + 1) * 512], bo0_v[:, k, :],
                        kyx_v[:, k, hb * 512:(hb + 1) * 512],
                        start=(k == 0), stop=False, skip_group_check=True)

            # ================= iterations =================
            slots = None
            for it in range(NUM_ITER):
                last = it == NUM_ITER - 1

                # ---- z-mix of the other 7 slices -> brt ----
                brt = wp.tile([128, FB], f16, tag="brt")
                if it == 0:
                    qsrc = sb_q0[:, :].rearrange("p (j f) -> p j f", j=NC)
                    zmix = sb_zcor0
                    ds = list(range(1, NC))
                else:
                    qsrc = slots[:, :].rearrange("p (d f) -> p d f", d=NC)
                    zmix = sb_zcor
                    ds = list(range(NC))
                nc.vector.tensor_scalar_mul(
                    brt[:, :], qsrc[:, ds[0], 0:FB], zmix[:, ds[0]:ds[0] + 1])
                for d in ds[1:]:
                    nc.vector.scalar_tensor_tensor(
                        brt[:, :], qsrc[:, d, 0:FB], zmix[:, d:d + 1],
                        brt[:, :], OP.mult, OP.add)

                # ---- spatial rest (accumulates onto the prestart) ----
                brt_v = brt[:, :].rearrange("p (t l) -> p t l", l=L)
                for k in range(NT):
                    for hb in range(2):
                        nc.tensor.matmul(
                            sp_cur[:, hb * 512:(hb + 1) * 512], brt_v[:, k, :],
                            kyx_v[:, k, hb * 512:(hb + 1) * 512],
                            start=False, stop=(k == NT - 1),
                            skip_group_check=True)

                # ---- P reduce over gathered partials ----
                sbP = wp.tile([NF, L], f16, tag="sbp")
                if it == 0:
                    pv = slots0[:, :].rearrange("p (d f) -> p d f", d=NC)
                    pc = 0
                else:
                    pv = slots[0:NF, :].rearrange("p (d f) -> p d f", d=NC)
                    pc = FB
                t1 = wp.tile([NF, 4 * L], f32, tag="pt1")
                t1v = t1[:, :].rearrange("p (a l) -> p a l", l=L)
                nc.vector.tensor_tensor(
                    t1v, pv[:, 0:4, pc:pc + L], pv[:, 4:8, pc:pc + L], OP.add)
                t2 = wp.tile([NF, 2 * L], f32, tag="pt2")
                t2v = t2[:, :].rearrange("p (a l) -> p a l", l=L)
                nc.vector.tensor_tensor(t2v, t1v[:, 0:2, :], t1v[:, 2:4, :], OP.add)
                nc.vector.tensor_tensor(sbP[:, :], t2v[:, 0, :], t2v[:, 1, :], OP.add)

                # ---- bilateral message: P^T @ featc (norm folded in) ----
                bil = ps_bil_p.tile([L, NYX], f32, tag="bil")
                for hb in range(2):
                    nc.tensor.matmul(
                        bil[:, hb * 512:(hb + 1) * 512], sbP[:, :],
                        sb_featc[:, hb * 512:(hb + 1) * 512],
                        start=True, stop=True)

                # ---- cat assembly (ACT/DVE split, disjoint PSUM banks) ----
                nc.scalar.copy(cats[0][0:L, :], bil[:, 0:512])
                nc.vector.tensor_copy(cats[1][0:L, :], bil[:, 512:NYX])
                nc.vector.tensor_copy(cats[0][32:53, :], sp_cur[:, 0:512])
                nc.scalar.copy(cats[1][32:53, :], sp_cur[:, 512:NYX])

                # ---- mixing matmul (+unary via identity rows) ----
                psct = ps_ct_p.tile([128, FB], f32, tag="ct")
                for g in range(NT):
                    h, tl = g // 4, g % 4
                    nc.tensor.matmul(
                        psct[:, g * L:(g + 1) * L],
                        cats[h][:, tl * 128:(tl + 1) * 128],
                        sb_w12t[:, :], start=True, stop=True)

                # ---- softmax ----
                ex = wp.tile([128, FB], f32, tag="ex")
                nc.scalar.activation(ex[:, :], psct[:, :], AF.Exp)
                ex_v = ex[:, :].rearrange("p (t l) -> p t l", l=L)
                red = wp.tile([128, NT], f32, tag="red")
                nc.vector.tensor_reduce(red[:, :], ex_v, mybir.AxisListType.X, OP.add)
                rcp = wp.tile([128, NT], f32, tag="rcp")
                nc.vector.reciprocal(rcp[:, :], red[:, :])
                if last:
                    out_v = sb_out[:, :].rearrange("p (t l) -> p t l", l=L)
                    nc.vector.tensor_tensor(
                        out_v, ex_v, rcp[:, :].broadcast_to([128, NT, L]), OP.mult)
                    nc.sync.dma_start(out_d[:, :], sb_out[:, :])
                else:
                    stag = stp.tile([128, CCW], f16, tag="stag")
                    q_v = stag[:, 0:FB].rearrange("p (t l) -> p t l", l=L)
                    nc.vector.tensor_tensor(
                        q_v, ex_v, rcp[:, :].broadcast_to([128, NT, L]), OP.mult)
                    # own P partial for the next iteration
                    psP = ps_p.tile([NF, L], f32, tag="pp")
                    for t in range(NT):
                        nc.tensor.matmul(
                            psP[:, :], sb_featrt[:, t * NF:(t + 1) * NF],
                            stag[:, t * L:(t + 1) * L],
                            start=(t == 0), stop=(t == NT - 1))
                    nc.scalar.copy(stag[0:NF, FB:FB + L], psP[:, :])
                    # own-slice z coefficient for the prestart
                    bo = wp.tile([128, FB], f16, tag="bo")
                    nc.vector.tensor_scalar_mul(
                        bo[:, :], stag[:, 0:FB], sb_zcoo[:, 0:1])
                    # one AllGather carries q_own + P_partial
                    cc_in = dp.tile([128, CCW], f16, tag="ccin")
                    cc_out = dp.tile([128 * NC, CCW], f16, tag="ccout")
                    nc.sync.dma_start(cc_in[:, :], stag[:, :])
                    nc.gpsimd.collective_compute(
                        "AllGather", mybir.AluOpType.bypass,
                        replica_groups=[list(range(NC))],
                        ins=[cc_in.opt()], outs=[cc_out.opt()],
                    )
                    # prestart next iteration's own-slice spatial during CC
                    sp_next = ps_sp.tile([L, NYX], f32, tag="spat")
                    bo_v = bo[:, :].rearrange("p (t l) -> p t l", l=L)
                    for k in range(NT):
                        for hb in range(2):
                            nc.tensor.matmul(
                                sp_next[:, hb * 512:(hb + 1) * 512],
                                bo_v[:, k, :],
                                kyx_v[:, k, hb * 512:(hb + 1) * 512],
                                start=(k == 0), stop=False,
                                skip_group_check=True)
                    # unload gathered q + P partials
                    slots = slp.tile([128, NC * CCW], f16, tag="slots")
                    nc.sync.dma_start(
                        slots[:, :].rearrange("p (d f) -> p d f", d=NC),
                        cc_out[:, :].rearrange("(d p) f -> p d f", p=128))
                    sp_cur = sp_next
    nc.compile()
    return nc


def _host_prep(image, logits):
    """Per-core input dicts. q0 blocks are permuted own-slice-first."""
    img = np.asarray(image, dtype=np.float32)[0]      # [3, D, H, W]
    lg = np.asarray(logits, dtype=np.float32)[0]      # [L, D, H, W]

    zz, yy, xx = np.meshgrid(
        np.arange(D), np.arange(H), np.arange(W), indexing="ij")
    pos = np.stack([zz, yy, xx], -1).reshape(N, 3).astype(np.float32)
    rgb = img.reshape(3, N).T
    feat = np.concatenate([pos / ALPHA, rgb / BETA], axis=1).astype(np.float16)
    f = feat.astype(np.float32)                       # [N, 6] fp16-rounded
    sq = np.sum(f * f, axis=1)
    al = PA / 2 - sq / 2                              # alpha == beta per formula

    pairs = [(a, b) for a in range(6) for b in range(a + 1, 6)]

    def mono_row(ff, alv):
        cols = [PC * alv * alv, PC * np.ones_like(alv), PC * 2 * alv]
        cols += [PC * 2 * alv * ff[:, a] for a in range(6)]
        cols += [PC * 2 * ff[:, a] for a in range(6)]
        cols += [PC * ff[:, a] ** 2 for a in range(6)]
        cols += [PC * 2 * ff[:, a] * ff[:, b] for a, b in pairs]
        return np.stack(cols, 0)                      # [36, n]

    def mono_col(ff, bev):
        cols = [np.ones_like(bev), bev * bev, bev]
        cols += [ff[:, a] for a in range(6)]
        cols += [bev * ff[:, a] for a in range(6)]
        cols += [ff[:, a] ** 2 for a in range(6)]
        cols += [ff[:, a] * ff[:, b] for a, b in pairs]
        return np.stack(cols, 0)                      # [36, n]

    featr = mono_row(f, al).astype(np.float16)        # [36, N]
    rsum = featr.astype(np.float32).sum(axis=1)       # [36] for the normalizer

    r1 = np.arange(D, dtype=np.float32)
    Gz = np.exp(-0.5 * ((r1[:, None] - r1[None, :]) / GAMMA) ** 2)
    r2 = np.arange(H, dtype=np.float32)
    Gy = np.exp(-0.5 * ((r2[:, None] - r2[None, :]) / GAMMA) ** 2)
    Kyx = np.kron(Gy, Gy).astype(np.float32)          # H == W so Gy == Gx
    nyx = Kyx.sum(axis=0)
    Kyx_n = (Kyx / nyx[None, :]).astype(np.float16)
    czsum = Gz.sum(axis=0)
    kyx_in = np.ascontiguousarray(
        Kyx_n.reshape(NT, 128, NYX).transpose(1, 0, 2).reshape(128, NT * NYX))

    unary = lg.reshape(L, N)
    # voxel-major blocks: blkT[p, d, t*L+l] = unary[l, d*NYX + t*128 + p]
    blkT = unary.astype(np.float16).reshape(L, D, NT, 128).transpose(3, 1, 2, 0)

    maps = []
    for r in range(NC):
        sl = slice(r * NYX, (r + 1) * NYX)
        featc = mono_col(f[sl], al[sl]).astype(np.float16)      # [36, 1024]
        norm = rsum @ featc.astype(np.float32)                  # [1024]
        featc_n = (featc.astype(np.float32)
                   * (FSCALE / norm)[None, :]).astype(np.float16)
        featrt = np.ascontiguousarray(
            featr[:, sl].reshape(NF, NT, 128).transpose(2, 1, 0)
            .reshape(128, NT * NF))
        perm = [r] + [d for d in range(NC) if d != r]
        un = np.ascontiguousarray(blkT[:, perm].reshape(128, NC * FB))
        zvec = (Gz[:, r] / czsum[r]).astype(np.float32)
        zcor = zvec.copy()
        zcor[r] = 0.0
        zcor0 = zvec[perm].copy()
        zcor0[0] = 0.0
        unownt = np.ascontiguousarray(unary[:, sl].astype(np.float16))
        maps.append({
            "unaryt": un,
            "featrt": featrt,
            "featc": np.ascontiguousarray(featc_n),
            "kyx": kyx_in,
            "zcor": np.ascontiguousarray(np.tile(zcor, (128, 1))),
            "zcor0": np.ascontiguousarray(np.tile(zcor0, (128, 1))),
            "zcoo": np.full((128, 1), zvec[r], np.float32),
            "unownt": unownt,
        })
    return maps


def _host_w12t(spatial_ker_weights, bilateral_ker_weights, compatibility_matrix):
    ws = np.asarray(spatial_ker_weights, np.float32)
    wb = np.asarray(bilateral_ker_weights, np.float32)
    cm = np.asarray(compatibility_matrix, np.float32)
    w = np.zeros((CATP, L), np.float32)
    w[0:L] = (cm @ wb).T / FSCALE
    w[32:53] = (cm @ ws).T
    w[64:CATP] = np.eye(L)
    return np.ascontiguousarray(w.astype(np.float16))


def kernel(image, logits, spatial_ker_weights, bilateral_ker_weights,
           compatibility_matrix):
    from concourse.bass_utils import run_bass_kernel_spmd

    if "nc" not in _CACHE:
        _CACHE["nc"] = _build_nc()
    nc = _CACHE["nc"]

    maps = _host_prep(image, logits)
    w12t = _host_w12t(spatial_ker_weights, bilateral_ker_weights,
                      compatibility_matrix)
    for m in maps:
        m["w12t"] = w12t

    res = run_bass_kernel_spmd(nc, maps, core_ids=list(range(NC)))

    out = np.empty((L, D, H, W), dtype=np.float32)
    for r in range(NC):
        blk = res.results[r]["out"]                   # [128, 168]
        out[:, r] = blk.reshape(128, NT, L).transpose(2, 1, 0).reshape(L, H, W)
    return out[None]


# revision 11
# speedup vs baseline: 1.3934x; 1.3934x over previous
"""CRF-RNN (dense Gaussian CRF mean-field) Trainium2 kernel, 8 NeuronCores.

Strategy (v3 — factorized bilateral, latency-tuned)
---------------------------------------------------
N = 8*32*32 = 8192 voxels, L = 21 labels, 5 mean-field iterations.
- Bilateral kernel K_b is rank-36 (quadratic exp fit PC*(PA-d^2/2)^2 as a
  36-dim feature map), so q@K_b = (q@R^T)@C with P = q@R^T a [36,21]
  matrix; K_b is never materialized. Each core ships its own-slice P
  partial inside the per-iteration AllGather; the bilateral message is
  2 matmuls P^T @ featc (1/norm_b and the /FSCALE fold into featc/w12t).
- Spatial kernel is separable: z-mix across the 8 gathered slices, then
  one brt @ (Gy x Gx) pass (fp16, 16 matmuls). The own-slice term is
  prestarted during the AllGather; the other-slice z-mix runs on the PE
  as 8 scaled-identity matmuls (zid input) — the DVE was 3x slower.
- One fp16 AllGather per iteration: [q_own (168) | P partial (21)].
  q-part DMA to the CC buffer is issued right after softmax; the P part
  follows as a tiny second DMA.
- Unary folds into the mixing matmul (cat rows 64:85 = unown^T, w12t
  rows 64:85 = I), so softmax exp reads the mix PSUM directly.
- bil (rows 0:21) and spat (rows 32:53) share one PSUM tile; the cat
  assembly is 2 wide [53,512] copies split across ACT/DVE banks.
- ~18 dummy F=512 matmuls into a scratch PSUM bank after each collective
  keep the PE HAM-warm through the CC window (else every iteration's
  matmuls run at 1.2 GHz).
- Per-core q0/unary blocks are permuted own-slice-first so the SPMD
  program is core-independent; iter-0 P comes from a local 64-matmul
  contraction over the full permuted featrt (no collective needed).
"""

import numpy as np

ALPHA, BETA, GAMMA = 160.0, 3.0, 3.0
NUM_ITER = 5
L, D, H, W = 21, 8, 32, 32
NC = 8
NYX = H * W            # 1024
N = D * NYX            # 8192
NT = NYX // 128        # 8 chunks per slice
FB = NT * L            # 168  free width of one q block
NF = 36                # polynomial feature-map width
FSCALE = 1024.0        # featc pre-scale (folded back via w12t bil rows)
CCW = FB + 24          # 192: 168 q cols + 21 P cols + 3 pad
CATP = 85              # cat rows: 0:21 bil, 32:53 spat, 64:85 unownT
NWARM = 18             # PE warmer matmuls per collective window

PA = 2.105             # quadratic exp fit: exp(x) ~= PC*(x+PA)^2 on [-0.21,0]
PC = 0.22538087

_CACHE = {}


def _build_nc():
    import concourse.bass as bass  # noqa: F401
    import concourse.bacc as bacc
    import concourse.mybir as mybir
    import concourse.tile as tile
    import concourse.tile_utils as tile_utils

    try:
        tile_utils.max_sbuf_usage = 204 * 1024
    except Exception:
        pass

    f32 = mybir.dt.float32
    f16 = mybir.dt.float16
    AF = mybir.ActivationFunctionType
    OP = mybir.AluOpType

    nc = bacc.Bacc(None, target_bir_lowering=False, num_devices=NC)

    unary_d = nc.declare_dram_parameter("unaryt", [128, NC * FB], f16, isOutput=False)
    featrt_d = nc.declare_dram_parameter("featrt", [128, NC * NT * NF], f16, isOutput=False)
    featc_d = nc.declare_dram_parameter("featc", [NF, NYX], f16, isOutput=False)
    kyx_d = nc.declare_dram_parameter("kyx", [128, NT * NYX], f16, isOutput=False)
    zid_d = nc.declare_dram_parameter("zid", [128, NC * 128], f16, isOutput=False)
    zcor0_d = nc.declare_dram_parameter("zcor0", [128, NC], f32, isOutput=False)
    zcoo_d = nc.declare_dram_parameter("zcoo", [128, 1], f32, isOutput=False)
    w12t_d = nc.declare_dram_parameter("w12t", [CATP, L], f16, isOutput=False)
    unownt_d = nc.declare_dram_parameter("unownt", [L, NYX], f16, isOutput=False)
    out_d = nc.declare_dram_parameter("out", [128, FB], f32, isOutput=True)

    with tile.TileContext(nc) as tc:
        with (
            tc.tile_pool(name="persist", bufs=1) as pp,
            tc.tile_pool(name="work", bufs=4) as wp,
            tc.tile_pool(name="slots", bufs=2) as slp,
            tc.tile_pool(name="stag", bufs=2) as stp,
            tc.tile_pool(name="dram", bufs=2, space="DRAM") as dp,
            tc.tile_pool(name="ps_sp", bufs=1, space="PSUM") as ps_sp,
            tc.tile_pool(name="ps_ct", bufs=1, space="PSUM") as ps_ct_p,
            tc.tile_pool(name="ps_p", bufs=1, space="PSUM") as ps_p,
            tc.tile_pool(name="ps_brt", bufs=1, space="PSUM") as ps_brt_p,
            tc.tile_pool(name="ps_warm", bufs=1, space="PSUM") as ps_warm_p,
        ):
            sb_wu = pp.tile([128, 8], f16, tag="wu")
            sb_unary = pp.tile([128, NC * FB], f16, tag="unary")
            sb_featrt = pp.tile([128, NC * NT * NF], f16, tag="featrt")
            sb_featc = pp.tile([NF, NYX], f16, tag="featc")
            sb_kyx = pp.tile([128, NT * NYX], f16, tag="kyx")
            sb_zid = pp.tile([128, NC * 128], f16, tag="zid")
            sb_zcor0 = pp.tile([128, NC], f32, tag="zcor0")
            sb_zcoo = pp.tile([128, 1], f32, tag="zcoo")
            sb_w12t = pp.tile([CATP, L], f16, tag="w12t")
            sb_unt = pp.tile([L, NYX], f16, tag="unt")
            cats = [pp.tile([CATP, 512], f16, tag=f"cat{h}", name=f"cat{h}")
                    for h in range(2)]
            sb_ex0 = pp.tile([128, NC * FB], f32, tag="ex0")
            sb_red0 = pp.tile([128, NC * NT], f32, tag="red0")
            sb_rcp0 = pp.tile([128, NC * NT], f32, tag="rcp0")
            sb_q0 = pp.tile([128, NC * FB], f16, tag="q0")
            sb_out = pp.tile([128, FB], f32, tag="outt")
            sb_warm = pp.tile([128, 1], f32, tag="warm")
            sb_warm2 = pp.tile([128, 1], f32, tag="warm2")

            # collective warmup FIRST: its DMA heads the sync queue so the
            # trigger fires ASAP and ncfw init overlaps all of setup+iter0.
            nc.vector.memset(sb_wu[:, :], 0.0)
            wu_in = dp.tile([128, 8], f16, tag="wuin")
            wu_out = dp.tile([128 * NC, 8], f16, tag="wuout")
            nc.sync.dma_start(wu_in[:, :], sb_wu[:, :])
            nc.gpsimd.collective_compute(
                "AllGather", mybir.AluOpType.bypass,
                replica_groups=[list(range(NC))],
                ins=[wu_in.opt()], outs=[wu_out.opt()],
            )

            nc.sync.dma_start(sb_unary[:, :], unary_d[:, :])
            nc.sync.dma_start(sb_featrt[:, :], featrt_d[:, :])
            nc.sync.dma_start(sb_featc[:, :], featc_d[:, :])
            nc.sync.dma_start(sb_zcor0[:, :], zcor0_d[:, :])
            nc.sync.dma_start(sb_zcoo[:, :], zcoo_d[:, :])
            nc.sync.dma_start(sb_w12t[:, :], w12t_d[:, :])
            nc.sync.dma_start(sb_unt[:, :], unownt_d[:, :])
            nc.sync.dma_start(sb_zid[:, :], zid_d[:, :])
            nc.sync.dma_start(sb_kyx[:, :], kyx_d[:, :])

            # exp table prewarm (~2.7us, overlaps DMA)
            nc.vector.memset(sb_warm[:, :], 0.0)
            nc.scalar.activation(sb_warm2[:, :], sb_warm[:, :], AF.Exp)

            # persistent PSUM: two bil+spat tiles (ping-pong), memset once
            # so the never-matmul-written rows 21:32 can't carry NaN junk.
            sp_tiles = [ps_sp.tile([53, NYX], f32, tag=f"spat{j}",
                                   name=f"spat{j}") for j in range(2)]
            for j in range(2):
                nc.vector.memset(sp_tiles[j][:, :], 0.0)
            ps_warm = ps_warm_p.tile([L, 512], f32, tag="warmp")

            # cat gap rows + unownT rows
            for h in range(2):
                nc.vector.memset(cats[h][:, :], 0.0)
            nc.vector.tensor_copy(cats[0][64:CATP, :], sb_unt[:, 0:512])
            nc.scalar.copy(cats[1][64:CATP, :], sb_unt[:, 512:NYX])

            kyx_v = sb_kyx[:, :].rearrange("p (k c) -> p k c", c=NYX)

            # ---------------- iter-0 softmax over full permuted unary ----
            ex0_v = sb_ex0[:, :].rearrange("p (g l) -> p g l", l=L)
            q0_v = sb_q0[:, :].rearrange("p (g l) -> p g l", l=L)
            HW2 = NC * FB // 2
            G2 = NC * NT // 2
            for h in range(2):
                cs = slice(h * HW2, (h + 1) * HW2)
                gs = slice(h * G2, (h + 1) * G2)
                nc.scalar.activation(sb_ex0[:, cs], sb_unary[:, cs], AF.Exp)
                nc.vector.tensor_reduce(
                    sb_red0[:, gs], ex0_v[:, gs, :], mybir.AxisListType.X, OP.add)
                nc.vector.reciprocal(sb_rcp0[:, gs], sb_red0[:, gs])
                nc.vector.tensor_tensor(
                    q0_v[:, gs, :], ex0_v[:, gs, :],
                    sb_rcp0[:, gs].broadcast_to([128, G2, L]), OP.mult)

            # ---------------- iter-0 P over all 64 (permuted) chunks ----
            psP0 = ps_p.tile([NF, L], f32, tag="pp")
            for g in range(NC * NT):
                nc.tensor.matmul(
                    psP0[:, :], sb_featrt[:, g * NF:(g + 1) * NF],
                    sb_q0[:, g * L:(g + 1) * L],
                    start=(g == 0), stop=(g == NC * NT - 1))

            # prestart iter-0 own-slice spatial (overlaps kyx DMA tail)
            bo0 = wp.tile([128, FB], f16, tag="bo")
            nc.vector.tensor_scalar_mul(bo0[:, :], sb_q0[:, 0:FB], sb_zcoo[:, 0:1])
            bo0_v = bo0[:, :].rearrange("p (t l) -> p t l", l=L)
            sp_cur = sp_tiles[0]
            for k in range(NT):
                for hb in range(2):
                    nc.tensor.matmul(
                        sp_cur[32:53, hb * 512:(hb + 1) * 512], bo0_v[:, k, :],
                        kyx_v[:, k, hb * 512:(hb + 1) * 512],
                        start=(k == 0), stop=False, skip_group_check=True)

            # ================= iterations =================
            slots = None
            bo = bo0
            for it in range(NUM_ITER):
                last = it == NUM_ITER - 1

                # ---- z-mix of the other 7 slices -> brt (fp16 SBUF) ----
                brt = wp.tile([128, FB], f16, tag="brt")
                if it == 0:
                    # local q0, DVE path (runs during the first-CC wait)
                    qsrc = sb_q0[:, :].rearrange("p (j f) -> p j f", j=NC)
                    nc.vector.tensor_scalar_mul(
                        brt[:, :], qsrc[:, 1, 0:FB], sb_zcor0[:, 1:2])
                    for d in range(2, NC):
                        nc.vector.scalar_tensor_tensor(
                            brt[:, :], qsrc[:, d, 0:FB], sb_zcor0[:, d:d + 1],
                            brt[:, :], OP.mult, OP.add)
                else:
                    # gathered slots, PE path: 8 scaled-identity matmuls
                    ps_brt = ps_brt_p.tile([128, FB], f32, tag="brtp")
                    for d in range(NC):
                        nc.tensor.matmul(
                            ps_brt[:, :], sb_zid[:, d * 128:(d + 1) * 128],
                            slots[:, d * CCW:d * CCW + FB],
                            start=(d == 0), stop=(d == NC - 1))
                    nc.vector.tensor_copy(brt[:, :], ps_brt[:, :])

                # ---- spatial rest (accumulates onto the prestart) ----
                brt_v = brt[:, :].rearrange("p (t l) -> p t l", l=L)
                for k in range(NT):
                    for hb in range(2):
                        nc.tensor.matmul(
                            sp_cur[32:53, hb * 512:(hb + 1) * 512], brt_v[:, k, :],
                            kyx_v[:, k, hb * 512:(hb + 1) * 512],
                            start=False, stop=(k == NT - 1),
                            skip_group_check=True)

                # ---- P: reduce gathered partials (or copy local P0) ----
                sbP = wp.tile([NF, L], f16, tag="sbp")
                if it == 0:
                    nc.scalar.copy(sbP[:, :], psP0[:, :])
                else:
                    pv = slots[0:NF, :].rearrange("p (d f) -> p d f", d=NC)
                    t1 = wp.tile([NF, 4 * L], f32, tag="pt1")
                    t1v = t1[:, :].rearrange("p (a l) -> p a l", l=L)
                    nc.vector.tensor_tensor(
                        t1v, pv[:, 0:4, FB:FB + L], pv[:, 4:8, FB:FB + L], OP.add)
                    t2 = wp.tile([NF, 2 * L], f32, tag="pt2")
                    t2v = t2[:, :].rearrange("p (a l) -> p a l", l=L)
                    nc.vector.tensor_tensor(t2v, t1v[:, 0:2, :], t1v[:, 2:4, :], OP.add)
                    nc.vector.tensor_tensor(sbP[:, :], t2v[:, 0, :], t2v[:, 1, :], OP.add)

                # ---- bilateral message into rows 0:21 of the same tile ----
                for hb in range(2):
                    nc.tensor.matmul(
                        sp_cur[0:L, hb * 512:(hb + 1) * 512], sbP[:, :],
                        sb_featc[:, hb * 512:(hb + 1) * 512],
                        start=True, stop=True, skip_group_check=True)

                # ---- cat assembly: 2 wide copies, ACT/DVE on separate banks
                nc.scalar.copy(cats[0][0:53, :], sp_cur[:, 0:512])
                nc.vector.tensor_copy(cats[1][0:53, :], sp_cur[:, 512:NYX])

                # ---- mixing matmul (+unary via identity rows) ----
                psct = ps_ct_p.tile([128, FB], f32, tag="ct")
                for g in range(NT):
                    h, tl = g // 4, g % 4
                    nc.tensor.matmul(
                        psct[:, g * L:(g + 1) * L],
                        cats[h][:, tl * 128:(tl + 1) * 128],
                        sb_w12t[:, :], start=True, stop=True)

                # ---- softmax ----
                ex = wp.tile([128, FB], f32, tag="ex")
                nc.scalar.activation(ex[:, :], psct[:, :], AF.Exp)
                ex_v = ex[:, :].rearrange("p (t l) -> p t l", l=L)
                red = wp.tile([128, NT], f32, tag="red")
                nc.vector.tensor_reduce(red[:, :], ex_v, mybir.AxisListType.X, OP.add)
                rcp = wp.tile([128, NT], f32, tag="rcp")
                nc.vector.reciprocal(rcp[:, :], red[:, :])
                if last:
                    out_v = sb_out[:, :].rearrange("p (t l) -> p t l", l=L)
                    nc.vector.tensor_tensor(
                        out_v, ex_v, rcp[:, :].broadcast_to([128, NT, L]), OP.mult)
                    nc.sync.dma_start(out_d[:, :], sb_out[:, :])
                else:
                    stag = stp.tile([128, CCW], f16, tag="stag")
                    q_v = stag[:, 0:FB].rearrange("p (t l) -> p t l", l=L)
                    nc.vector.tensor_tensor(
                        q_v, ex_v, rcp[:, :].broadcast_to([128, NT, L]), OP.mult)
                    cc_in = dp.tile([128, CCW], f16, tag="ccin")
                    cc_out = dp.tile([128 * NC, CCW], f16, tag="ccout")
                    # ship the q part immediately; P follows as a tiny DMA
                    nc.sync.dma_start(cc_in[:, 0:FB], stag[:, 0:FB])
                    psP = ps_p.tile([NF, L], f32, tag="pp")
                    for t in range(NT):
                        nc.tensor.matmul(
                            psP[:, :], sb_featrt[:, t * NF:(t + 1) * NF],
                            stag[:, t * L:(t + 1) * L],
                            start=(t == 0), stop=(t == NT - 1))
                    nc.scalar.copy(stag[0:NF, FB:FB + L], psP[:, :])
                    nc.sync.dma_start(cc_in[0:NF, FB:FB + L], stag[0:NF, FB:FB + L])
                    bo = wp.tile([128, FB], f16, tag="bo")
                    nc.vector.tensor_scalar_mul(
                        bo[:, :], stag[:, 0:FB], sb_zcoo[:, 0:1])
                    nc.gpsimd.collective_compute(
                        "AllGather", mybir.AluOpType.bypass,
                        replica_groups=[list(range(NC))],
                        ins=[cc_in.opt()], outs=[cc_out.opt()],
                    )
                    # prestart next iteration's own-slice spatial during CC
                    sp_next = sp_tiles[(it + 1) % 2]
                    bo_v = bo[:, :].rearrange("p (t l) -> p t l", l=L)
                    for k in range(NT):
                        for hb in range(2):
                            nc.tensor.matmul(
                                sp_next[32:53, hb * 512:(hb + 1) * 512],
                                bo_v[:, k, :],
                                kyx_v[:, k, hb * 512:(hb + 1) * 512],
                                start=(k == 0), stop=False,
                                skip_group_check=True)
                    # HAM warmers: keep the PE busy through the CC window
                    for wmi in range(NWARM):
                        nc.tensor.matmul(
                            ps_warm[:, :], bo_v[:, wmi % NT, :],
                            kyx_v[:, wmi % NT, 0:512],
                            start=True, stop=True, skip_group_check=True)
                    # unload gathered q + P partials
                    slots = slp.tile([128, NC * CCW], f16, tag="slots")
                    nc.sync.dma_start(
                        slots[:, :].rearrange("p (d f) -> p d f", d=NC),
                        cc_out[:, :].rearrange("(d p) f -> p d f", p=128))
                    sp_cur = sp_next
    nc.compile()
    return nc


def _host_prep(image, logits):
    """Per-core input dicts. q0/featrt blocks are permuted own-slice-first."""
    img = np.asarray(image, dtype=np.float32)[0]      # [3, D, H, W]
    lg = np.asarray(logits, dtype=np.float32)[0]      # [L, D, H, W]

    zz, yy, xx = np.meshgrid(
        np.arange(D), np.arange(H), np.arange(W), indexing="ij")
    pos = np.stack([zz, yy, xx], -1).reshape(N, 3).astype(np.float32)
    rgb = img.reshape(3, N).T
    feat = np.concatenate([pos / ALPHA, rgb / BETA], axis=1).astype(np.float16)
    f = feat.astype(np.float32)                       # [N, 6] fp16-rounded
    sq = np.sum(f * f, axis=1)
    al = PA / 2 - sq / 2                              # alpha == beta per formula

    pairs = [(a, b) for a in range(6) for b in range(a + 1, 6)]

    def mono_row(ff, alv):
        cols = [PC * alv * alv, PC * np.ones_like(alv), PC * 2 * alv]
        cols += [PC * 2 * alv * ff[:, a] for a in range(6)]
        cols += [PC * 2 * ff[:, a] for a in range(6)]
        cols += [PC * ff[:, a] ** 2 for a in range(6)]
        cols += [PC * 2 * ff[:, a] * ff[:, b] for a, b in pairs]
        return np.stack(cols, 0)                      # [36, n]

    def mono_col(ff, bev):
        cols = [np.ones_like(bev), bev * bev, bev]
        cols += [ff[:, a] for a in range(6)]
        cols += [bev * ff[:, a] for a in range(6)]
        cols += [ff[:, a] ** 2 for a in range(6)]
        cols += [ff[:, a] * ff[:, b] for a, b in pairs]
        return np.stack(cols, 0)                      # [36, n]

    featr = mono_row(f, al).astype(np.float16)        # [36, N]
    rsum = featr.astype(np.float32).sum(axis=1)       # [36] for the normalizer
    # voxel-major feature chunks: [128, d, t, f]
    featrt_blk = featr.reshape(NF, D, NT, 128).transpose(3, 1, 2, 0)

    r1 = np.arange(D, dtype=np.float32)
    Gz = np.exp(-0.5 * ((r1[:, None] - r1[None, :]) / GAMMA) ** 2)
    r2 = np.arange(H, dtype=np.float32)
    Gy = np.exp(-0.5 * ((r2[:, None] - r2[None, :]) / GAMMA) ** 2)
    Kyx = np.kron(Gy, Gy).astype(np.float32)          # H == W so Gy == Gx
    nyx = Kyx.sum(axis=0)
    Kyx_n = (Kyx / nyx[None, :]).astype(np.float16)
    czsum = Gz.sum(axis=0)
    kyx_in = np.ascontiguousarray(
        Kyx_n.reshape(NT, 128, NYX).transpose(1, 0, 2).reshape(128, NT * NYX))

    unary = lg.reshape(L, N)
    # voxel-major blocks: blkT[p, d, t*L+l] = unary[l, d*NYX + t*128 + p]
    blkT = unary.astype(np.float16).reshape(L, D, NT, 128).transpose(3, 1, 2, 0)

    eye = np.eye(128, dtype=np.float32)
    maps = []
    for r in range(NC):
        sl = slice(r * NYX, (r + 1) * NYX)
        featc = mono_col(f[sl], al[sl]).astype(np.float16)      # [36, 1024]
        norm = rsum @ featc.astype(np.float32)                  # [1024]
        featc_n = (featc.astype(np.float32)
                   * (FSCALE / norm)[None, :]).astype(np.float16)
        perm = [r] + [d for d in range(NC) if d != r]
        un = np.ascontiguousarray(blkT[:, perm].reshape(128, NC * FB))
        featrt = np.ascontiguousarray(
            featrt_blk[:, perm].reshape(128, NC * NT * NF))
        zvec = (Gz[:, r] / czsum[r]).astype(np.float32)
        zcor0 = zvec[perm].copy()
        zcor0[0] = 0.0
        zcor = zvec.copy()
        zcor[r] = 0.0                                 # own slice via prestart
        zid = np.ascontiguousarray(
            (eye[None, :, :] * zcor[:, None, None])
            .transpose(1, 0, 2).reshape(128, NC * 128).astype(np.float16))
        unownt = np.ascontiguousarray(unary[:, sl].astype(np.float16))
        maps.append({
            "unaryt": un,
            "featrt": featrt,
            "featc": np.ascontiguousarray(featc_n),
            "kyx": kyx_in,
            "zid": zid,
            "zcor0": np.ascontiguousarray(np.tile(zcor0, (128, 1))),
            "zcoo": np.full((128, 1), zvec[r], np.float32),
            "unownt": unownt,
        })
    return maps


def _host_w12t(spatial_ker_weights, bilateral_ker_weights, compatibility_matrix):
    ws = np.asarray(spatial_ker_weights, np.float32)
    wb = np.asarray(bilateral_ker_weights, np.float32)
    cm = np.asarray(compatibility_matrix, np.float32)
    w = np.zeros((CATP, L), np.float32)
    w[0:L] = (cm @ wb).T / FSCALE
    w[32:53] = (cm @ ws).T
    w[64:CATP] = np.eye(L)
    return np.ascontiguousarray(w.astype(np.float16))


def kernel(image, logits, spatial_ker_weights, bilateral_ker_weights,
           compatibility_matrix):
    from concourse.bass_utils import run_bass_kernel_spmd

    if "nc" not in _CACHE:
        _CACHE["nc"] = _build_nc()
    nc = _CACHE["nc"]

    maps = _host_prep(image, logits)
    w12t = _host_w12t(spatial_ker_weights, bilateral_ker_weights,
                      compatibility_matrix)
    for m in maps:
        m["w12t"] = w12t

    res = run_bass_kernel_spmd(nc, maps, core_ids=list(range(NC)))

    out = np.empty((L, D, H, W), dtype=np.float32)
    for r in range(NC):
        blk = res.results[r]["out"]                   # [128, 168]
        out[:, r] = blk.reshape(128, NT, L).transpose(2, 1, 0).reshape(L, H, W)
    return out[None]


# revision 17
# speedup vs baseline: 1.4219x; 1.0205x over previous
"""CRF-RNN (dense Gaussian CRF mean-field) Trainium2 kernel, 8 NeuronCores.

Strategy (v3 — factorized bilateral, latency-tuned)
---------------------------------------------------
N = 8*32*32 = 8192 voxels, L = 21 labels, 5 mean-field iterations.
- Bilateral kernel K_b is rank-36 (quadratic exp fit PC*(PA-d^2/2)^2 as a
  36-dim feature map), so q@K_b = (q@R^T)@C with P = q@R^T a [36,21]
  matrix; K_b is never materialized. Each core ships its own-slice P
  partial inside the per-iteration AllGather; the bilateral message is
  2 matmuls P^T @ featc (1/norm_b and the /FSCALE fold into featc/w12t).
- Spatial kernel is separable: z-mix across the 8 gathered slices, then
  one brt @ (Gy x Gx) pass (fp16, 16 matmuls). The own-slice term is
  prestarted during the AllGather; the other-slice z-mix runs on the PE
  as 8 scaled-identity matmuls (zid input) — the DVE was 3x slower.
- One fp16 AllGather per iteration: [q_own (168) | P partial (21)].
  q-part DMA to the CC buffer is issued right after softmax; the P part
  follows as a tiny second DMA.
- Unary folds into the mixing matmul (cat rows 64:85 = unown^T, w12t
  rows 64:85 = I), so softmax exp reads the mix PSUM directly.
- bil (rows 0:21) and spat (rows 32:53) share one PSUM tile; the cat
  assembly is 2 wide [53,512] copies split across ACT/DVE banks.
- ~18 dummy F=512 matmuls into a scratch PSUM bank after each collective
  keep the PE HAM-warm through the CC window (else every iteration's
  matmuls run at 1.2 GHz).
- Per-core q0/unary blocks are permuted own-slice-first so the SPMD
  program is core-independent; iter-0 P comes from a local 64-matmul
  contraction over the full permuted featrt (no collective needed).
"""

import numpy as np

ALPHA, BETA, GAMMA = 160.0, 3.0, 3.0
NUM_ITER = 5
L, D, H, W = 21, 8, 32, 32
NC = 8
NYX = H * W            # 1024
N = D * NYX            # 8192
NT = NYX // 128        # 8 chunks per slice
FB = NT * L            # 168  free width of one q block
NF = 36                # polynomial feature-map width
FSCALE = 1024.0        # featc pre-scale (folded back via w12t bil rows)
CCW = FB + 24          # 192: 168 q cols + 21 P cols + 3 pad
CATP = 85              # cat rows: 0:21 bil, 32:53 spat, 64:85 unownT
NWARM = 30             # PE warmer matmuls per collective window

PA = 2.105             # quadratic exp fit: exp(x) ~= PC*(x+PA)^2 on [-0.21,0]
PC = 0.22538087

_CACHE = {}


def _build_nc():
    import concourse.bass as bass  # noqa: F401
    import concourse.bacc as bacc
    import concourse.mybir as mybir
    import concourse.tile as tile
    import concourse.tile_utils as tile_utils

    try:
        tile_utils.max_sbuf_usage = 204 * 1024
    except Exception:
        pass

    f32 = mybir.dt.float32
    f16 = mybir.dt.float16
    AF = mybir.ActivationFunctionType
    OP = mybir.AluOpType

    nc = bacc.Bacc(None, target_bir_lowering=False, num_devices=NC)

    unary_d = nc.declare_dram_parameter("unaryt", [128, NC * FB], f16, isOutput=False)
    featrt_d = nc.declare_dram_parameter("featrt", [128, NC * NT * NF], f16, isOutput=False)
    featc_d = nc.declare_dram_parameter("featc", [NF, NYX], f16, isOutput=False)
    kyx_d = nc.declare_dram_parameter("kyx", [128, NT * NYX], f16, isOutput=False)
    zid_d = nc.declare_dram_parameter("zid", [128, NC * 128], f16, isOutput=False)
    zcor0_d = nc.declare_dram_parameter("zcor0", [128, NC], f32, isOutput=False)
    zcoo_d = nc.declare_dram_parameter("zcoo", [128, 1], f32, isOutput=False)
    w12t_d = nc.declare_dram_parameter("w12t", [CATP, L], f16, isOutput=False)
    unownt_d = nc.declare_dram_parameter("unownt", [L, NYX], f16, isOutput=False)
    out_d = nc.declare_dram_parameter("out", [128, FB], f32, isOutput=True)

    with tile.TileContext(nc) as tc:
        with (
            tc.tile_pool(name="persist", bufs=1) as pp,
            tc.tile_pool(name="work", bufs=4) as wp,
            tc.tile_pool(name="slots", bufs=2) as slp,
            tc.tile_pool(name="stag", bufs=2) as stp,
            tc.tile_pool(name="dram", bufs=2, space="DRAM") as dp,
            tc.tile_pool(name="ps_sp", bufs=1, space="PSUM") as ps_sp,
            tc.tile_pool(name="ps_ct", bufs=1, space="PSUM") as ps_ct_p,
            tc.tile_pool(name="ps_p", bufs=1, space="PSUM") as ps_p,
            tc.tile_pool(name="ps_brt", bufs=1, space="PSUM") as ps_brt_p,
            tc.tile_pool(name="ps_warm", bufs=1, space="PSUM") as ps_warm_p,
        ):
            sb_wu = pp.tile([128, 8], f16, tag="wu")
            sb_unary = pp.tile([128, NC * FB], f16, tag="unary")
            sb_featrt = pp.tile([128, NC * NT * NF], f16, tag="featrt")
            sb_featc = pp.tile([NF, NYX], f16, tag="featc")
            sb_kyx = pp.tile([128, NT * NYX], f16, tag="kyx")
            sb_zid = pp.tile([128, NC * 128], f16, tag="zid")
            sb_zcor0 = pp.tile([128, NC], f32, tag="zcor0")
            sb_zcoo = pp.tile([128, 1], f32, tag="zcoo")
            sb_w12t = pp.tile([CATP, L], f16, tag="w12t")
            sb_unt = pp.tile([L, NYX], f16, tag="unt")
            cats = [pp.tile([CATP, 512], f16, tag=f"cat{h}", name=f"cat{h}")
                    for h in range(2)]
            sb_ex0 = pp.tile([128, NC * FB], f32, tag="ex0")
            sb_red0 = pp.tile([128, NC * NT], f32, tag="red0")
            sb_rcp0 = pp.tile([128, NC * NT], f32, tag="rcp0")
            sb_q0 = pp.tile([128, NC * FB], f16, tag="q0")
            sb_out = pp.tile([128, FB], f32, tag="outt")
            sb_warm = pp.tile([128, 1], f32, tag="warm")
            sb_warm2 = pp.tile([128, 1], f32, tag="warm2")

            # collective warmup FIRST: its DMA heads the sync queue so the
            # trigger fires ASAP and ncfw init overlaps all of setup+iter0.
            nc.vector.memset(sb_wu[:, :], 0.0)
            wu_in = dp.tile([128, 8], f16, tag="wuin")
            wu_out = dp.tile([128 * NC, 8], f16, tag="wuout")
            nc.sync.dma_start(wu_in[:, :], sb_wu[:, :])
            nc.gpsimd.collective_compute(
                "AllGather", mybir.AluOpType.bypass,
                replica_groups=[list(range(NC))],
                ins=[wu_in.opt()], outs=[wu_out.opt()],
            )

            nc.sync.dma_start(sb_unary[:, :], unary_d[:, :])
            nc.sync.dma_start(sb_featrt[:, :], featrt_d[:, :])
            nc.sync.dma_start(sb_featc[:, :], featc_d[:, :])
            nc.sync.dma_start(sb_zcor0[:, :], zcor0_d[:, :])
            nc.sync.dma_start(sb_zcoo[:, :], zcoo_d[:, :])
            nc.sync.dma_start(sb_w12t[:, :], w12t_d[:, :])
            nc.sync.dma_start(sb_unt[:, :], unownt_d[:, :])
            nc.sync.dma_start(sb_zid[:, :], zid_d[:, :])
            nc.sync.dma_start(sb_kyx[:, :], kyx_d[:, :])

            # exp table prewarm (~2.7us, overlaps DMA)
            nc.vector.memset(sb_warm[:, :], 0.0)
            nc.scalar.activation(sb_warm2[:, :], sb_warm[:, :], AF.Exp)

            # persistent PSUM: two bil+spat tile PAIRS (ping-pong), one tile
            # per 512-col half so the ACT/DVE cat copies hit independent
            # tiles (banks) and run concurrently. memset once so the
            # never-matmul-written rows 21:32 can't carry NaN junk.
            sp_tiles = [[ps_sp.tile([53, 512], f32, tag=f"spat{j}{hb}",
                                    name=f"spat{j}{hb}") for hb in range(2)]
                        for j in range(2)]
            for j in range(2):
                for hb in range(2):
                    nc.vector.memset(sp_tiles[j][hb][:, :], 0.0)
            ps_warm = ps_warm_p.tile([L, 512], f32, tag="warmp")

            # cat gap rows + unownT rows
            for h in range(2):
                nc.vector.memset(cats[h][:, :], 0.0)
            nc.vector.tensor_copy(cats[0][64:CATP, :], sb_unt[:, 0:512])
            nc.scalar.copy(cats[1][64:CATP, :], sb_unt[:, 512:NYX])

            kyx_v = sb_kyx[:, :].rearrange("p (k c) -> p k c", c=NYX)

            # ---------------- iter-0 softmax over full permuted unary ----
            ex0_v = sb_ex0[:, :].rearrange("p (g l) -> p g l", l=L)
            q0_v = sb_q0[:, :].rearrange("p (g l) -> p g l", l=L)
            HW2 = NC * FB // 2
            G2 = NC * NT // 2
            for h in range(2):
                cs = slice(h * HW2, (h + 1) * HW2)
                gs = slice(h * G2, (h + 1) * G2)
                nc.scalar.activation(sb_ex0[:, cs], sb_unary[:, cs], AF.Exp)
                nc.vector.tensor_reduce(
                    sb_red0[:, gs], ex0_v[:, gs, :], mybir.AxisListType.X, OP.add)
                nc.vector.reciprocal(sb_rcp0[:, gs], sb_red0[:, gs])
                nc.vector.tensor_tensor(
                    q0_v[:, gs, :], ex0_v[:, gs, :],
                    sb_rcp0[:, gs].broadcast_to([128, G2, L]), OP.mult)

            # ---------------- iter-0 P over all 64 (permuted) chunks ----
            psP0 = ps_p.tile([NF, L], f32, tag="pp")
            for g in range(NC * NT):
                nc.tensor.matmul(
                    psP0[:, :], sb_featrt[:, g * NF:(g + 1) * NF],
                    sb_q0[:, g * L:(g + 1) * L],
                    start=(g == 0), stop=(g == NC * NT - 1))

            # prestart iter-0 own-slice spatial (overlaps kyx DMA tail)
            bo0 = wp.tile([128, FB], f16, tag="bo")
            nc.vector.tensor_scalar_mul(bo0[:, :], sb_q0[:, 0:FB], sb_zcoo[:, 0:1])
            bo0_v = bo0[:, :].rearrange("p (t l) -> p t l", l=L)
            sp_cur = sp_tiles[0]
            for k in range(NT):
                for hb in range(2):
                    nc.tensor.matmul(
                        sp_cur[hb][32:53, :], bo0_v[:, k, :],
                        kyx_v[:, k, hb * 512:(hb + 1) * 512],
                        start=(k == 0), stop=False, skip_group_check=True)

            # ================= iterations =================
            slots = None
            bo = bo0
            for it in range(NUM_ITER):
                last = it == NUM_ITER - 1

                # ---- z-mix of the other 7 slices -> brt (fp16 SBUF) ----
                brt = wp.tile([128, FB], f16, tag="brt")
                if it == 0:
                    # local q0, DVE path (runs during the first-CC wait)
                    qsrc = sb_q0[:, :].rearrange("p (j f) -> p j f", j=NC)
                    nc.vector.tensor_scalar_mul(
                        brt[:, :], qsrc[:, 1, 0:FB], sb_zcor0[:, 1:2])
                    for d in range(2, NC):
                        nc.vector.scalar_tensor_tensor(
                            brt[:, :], qsrc[:, d, 0:FB], sb_zcor0[:, d:d + 1],
                            brt[:, :], OP.mult, OP.add)
                else:
                    # gathered slots, PE path: 8 scaled-identity matmuls,
                    # pipelined with the 8 per-rank unload DMAs
                    ps_brt = ps_brt_p.tile([128, FB], f32, tag="brtp")
                    for d in range(NC):
                        nc.tensor.matmul(
                            ps_brt[:, :], sb_zid[:, d * 128:(d + 1) * 128],
                            slots[:, d * CCW:d * CCW + FB],
                            start=(d == 0), stop=(d == NC - 1))
                    nc.vector.tensor_copy(brt[:, :], ps_brt[:, :])

                # ---- spatial rest (accumulates onto the prestart) ----
                brt_v = brt[:, :].rearrange("p (t l) -> p t l", l=L)
                for k in range(NT):
                    for hb in range(2):
                        nc.tensor.matmul(
                            sp_cur[hb][32:53, :], brt_v[:, k, :],
                            kyx_v[:, k, hb * 512:(hb + 1) * 512],
                            start=False, stop=(k == NT - 1),
                            skip_group_check=True)

                # ---- P: reduce gathered partials (or copy local P0) ----
                sbP = wp.tile([NF, L], f16, tag="sbp")
                if it == 0:
                    nc.scalar.copy(sbP[:, :], psP0[:, :])
                else:
                    pv = slots[0:NF, :].rearrange("p (d f) -> p d f", d=NC)
                    t1 = wp.tile([NF, 4 * L], f32, tag="pt1")
                    t1v = t1[:, :].rearrange("p (a l) -> p a l", l=L)
                    nc.vector.tensor_tensor(
                        t1v, pv[:, 0:4, FB:FB + L], pv[:, 4:8, FB:FB + L], OP.add)
                    t2 = wp.tile([NF, 2 * L], f32, tag="pt2")
                    t2v = t2[:, :].rearrange("p (a l) -> p a l", l=L)
                    nc.vector.tensor_tensor(t2v, t1v[:, 0:2, :], t1v[:, 2:4, :], OP.add)
                    nc.vector.tensor_tensor(sbP[:, :], t2v[:, 0, :], t2v[:, 1, :], OP.add)

                # ---- bilateral message into rows 0:21 of the same tiles ----
                for hb in range(2):
                    nc.tensor.matmul(
                        sp_cur[hb][0:L, :], sbP[:, :],
                        sb_featc[:, hb * 512:(hb + 1) * 512],
                        start=True, stop=True, skip_group_check=True)

                # ---- cat assembly: 2 wide copies, ACT/DVE on separate tiles
                nc.scalar.copy(cats[0][0:53, :], sp_cur[0][:, :])
                nc.vector.tensor_copy(cats[1][0:53, :], sp_cur[1][:, :])

                # ---- mixing matmul (+unary via identity rows) ----
                psct = ps_ct_p.tile([128, FB], f32, tag="ct")
                for g in range(NT):
                    h, tl = g // 4, g % 4
                    nc.tensor.matmul(
                        psct[:, g * L:(g + 1) * L],
                        cats[h][:, tl * 128:(tl + 1) * 128],
                        sb_w12t[:, :], start=True, stop=True)

                # ---- softmax ----
                ex = wp.tile([128, FB], f32, tag="ex")
                nc.scalar.activation(ex[:, :], psct[:, :], AF.Exp)
                ex_v = ex[:, :].rearrange("p (t l) -> p t l", l=L)
                red = wp.tile([128, NT], f32, tag="red")
                nc.vector.tensor_reduce(red[:, :], ex_v, mybir.AxisListType.X, OP.add)
                rcp = wp.tile([128, NT], f32, tag="rcp")
                nc.vector.reciprocal(rcp[:, :], red[:, :])
                if last:
                    out_v = sb_out[:, :].rearrange("p (t l) -> p t l", l=L)
                    nc.vector.tensor_tensor(
                        out_v, ex_v, rcp[:, :].broadcast_to([128, NT, L]), OP.mult)
                    nc.sync.dma_start(out_d[:, :], sb_out[:, :])
                else:
                    stag = stp.tile([128, CCW], f16, tag="stag")
                    q_v = stag[:, 0:FB].rearrange("p (t l) -> p t l", l=L)
                    nc.vector.tensor_tensor(
                        q_v, ex_v, rcp[:, :].broadcast_to([128, NT, L]), OP.mult)
                    cc_in = dp.tile([128, CCW], f16, tag="ccin")
                    cc_out = dp.tile([128 * NC, CCW], f16, tag="ccout")
                    # ship the q part immediately; P follows as a tiny DMA
                    nc.sync.dma_start(cc_in[:, 0:FB], stag[:, 0:FB])
                    psP = ps_p.tile([NF, L], f32, tag="pp")
                    for t in range(NT):
                        nc.tensor.matmul(
                            psP[:, :], sb_featrt[:, t * NF:(t + 1) * NF],
                            stag[:, t * L:(t + 1) * L],
                            start=(t == 0), stop=(t == NT - 1))
                    nc.scalar.copy(stag[0:NF, FB:FB + L], psP[:, :])
                    nc.sync.dma_start(cc_in[0:NF, FB:FB + L], stag[0:NF, FB:FB + L])
                    bo = wp.tile([128, FB], f16, tag="bo")
                    nc.vector.tensor_scalar_mul(
                        bo[:, :], stag[:, 0:FB], sb_zcoo[:, 0:1])
                    nc.gpsimd.collective_compute(
                        "AllGather", mybir.AluOpType.bypass,
                        replica_groups=[list(range(NC))],
                        ins=[cc_in.opt()], outs=[cc_out.opt()],
                    )
                    # prestart next iteration's own-slice spatial during CC
                    sp_next = sp_tiles[(it + 1) % 2]
                    bo_v = bo[:, :].rearrange("p (t l) -> p t l", l=L)
                    for k in range(NT):
                        for hb in range(2):
                            nc.tensor.matmul(
                                sp_next[hb][32:53, :],
                                bo_v[:, k, :],
                                kyx_v[:, k, hb * 512:(hb + 1) * 512],
                                start=(k == 0), stop=False,
                                skip_group_check=True)
                    # HAM warmers: keep the PE busy through the CC+unload
                    # window (a >3.4us PE idle re-throttles the clock)
                    for wmi in range(NWARM):
                        nc.tensor.matmul(
                            ps_warm[:, :], bo_v[:, wmi % NT, :],
                            kyx_v[:, wmi % NT, 0:512],
                            start=True, stop=True, skip_group_check=True)
                    # unload gathered q + P partials: one DMA per rank so
                    # the z-mix matmuls chain on per-rank arrival
                    slots = slp.tile([128, NC * CCW], f16, tag="slots")
                    for d in range(NC):
                        nc.sync.dma_start(
                            slots[:, d * CCW:(d + 1) * CCW],
                            cc_out[d * 128:(d + 1) * 128, :])
                    sp_cur = sp_next
    nc.compile()
    return nc


def _host_prep(image, logits):
    """Per-core input dicts. q0/featrt blocks are permuted own-slice-first."""
    img = np.asarray(image, dtype=np.float32)[0]      # [3, D, H, W]
    lg = np.asarray(logits, dtype=np.float32)[0]      # [L, D, H, W]

    zz, yy, xx = np.meshgrid(
        np.arange(D), np.arange(H), np.arange(W), indexing="ij")
    pos = np.stack([zz, yy, xx], -1).reshape(N, 3).astype(np.float32)
    rgb = img.reshape(3, N).T
    feat = np.concatenate([pos / ALPHA, rgb / BETA], axis=1).astype(np.float16)
    f = feat.astype(np.float32)                       # [N, 6] fp16-rounded
    sq = np.sum(f * f, axis=1)
    al = PA / 2 - sq / 2                              # alpha == beta per formula

    pairs = [(a, b) for a in range(6) for b in range(a + 1, 6)]

    def mono_row(ff, alv):
        cols = [PC * alv * alv, PC * np.ones_like(alv), PC * 2 * alv]
        cols += [PC * 2 * alv * ff[:, a] for a in range(6)]
        cols += [PC * 2 * ff[:, a] for a in range(6)]
        cols += [PC * ff[:, a] ** 2 for a in range(6)]
        cols += [PC * 2 * ff[:, a] * ff[:, b] for a, b in pairs]
        return np.stack(cols, 0)                      # [36, n]

    def mono_col(ff, bev):
        cols = [np.ones_like(bev), bev * bev, bev]
        cols += [ff[:, a] for a in range(6)]
        cols += [bev * ff[:, a] for a in range(6)]
        cols += [ff[:, a] ** 2 for a in range(6)]
        cols += [ff[:, a] * ff[:, b] for a, b in pairs]
        return np.stack(cols, 0)                      # [36, n]

    featr = mono_row(f, al).astype(np.float16)        # [36, N]
    rsum = featr.astype(np.float32).sum(axis=1)       # [36] for the normalizer
    # voxel-major feature chunks: [128, d, t, f]
    featrt_blk = featr.reshape(NF, D, NT, 128).transpose(3, 1, 2, 0)

    r1 = np.arange(D, dtype=np.float32)
    Gz = np.exp(-0.5 * ((r1[:, None] - r1[None, :]) / GAMMA) ** 2)
    r2 = np.arange(H, dtype=np.float32)
    Gy = np.exp(-0.5 * ((r2[:, None] - r2[None, :]) / GAMMA) ** 2)
    Kyx = np.kron(Gy, Gy).astype(np.float32)          # H == W so Gy == Gx
    nyx = Kyx.sum(axis=0)
    Kyx_n = (Kyx / nyx[None, :]).astype(np.float16)
    czsum = Gz.sum(axis=0)
    kyx_in = np.ascontiguousarray(
        Kyx_n.reshape(NT, 128, NYX).transpose(1, 0, 2).reshape(128, NT * NYX))

    unary = lg.reshape(L, N)
    # voxel-major blocks: blkT[p, d, t*L+l] = unary[l, d*NYX + t*128 + p]
    blkT = unary.astype(np.float16).reshape(L, D, NT, 128).transpose(3, 1, 2, 0)

    eye = np.eye(128, dtype=np.float32)
    maps = []
    for r in range(NC):
        sl = slice(r * NYX, (r + 1) * NYX)
        featc = mono_col(f[sl], al[sl]).astype(np.float16)      # [36, 1024]
        norm = rsum @ featc.astype(np.float32)                  # [1024]
        featc_n = (featc.astype(np.float32)
                   * (FSCALE / norm)[None, :]).astype(np.float16)
        perm = [r] + [d for d in range(NC) if d != r]
        un = np.ascontiguousarray(blkT[:, perm].reshape(128, NC * FB))
        featrt = np.ascontiguousarray(
            featrt_blk[:, perm].reshape(128, NC * NT * NF))
        zvec = (Gz[:, r] / czsum[r]).astype(np.float32)
        zcor0 = zvec[perm].copy()
        zcor0[0] = 0.0
        zcor = zvec.copy()
        zcor[r] = 0.0                                 # own slice via prestart
        zid = np.ascontiguousarray(
            (eye[None, :, :] * zcor[:, None, None])
            .transpose(1, 0, 2).reshape(128, NC * 128).astype(np.float16))
        unownt = np.ascontiguousarray(unary[:, sl].astype(np.float16))
        maps.append({
            "unaryt": un,
            "featrt": featrt,
            "featc": np.ascontiguousarray(featc_n),
            "kyx": kyx_in,
            "zid": zid,
            "zcor0": np.ascontiguousarray(np.tile(zcor0, (128, 1))),
            "zcoo": np.full((128, 1), zvec[r], np.float32),
            "unownt": unownt,
        })
    return maps


def _host_w12t(spatial_ker_weights, bilateral_ker_weights, compatibility_matrix):
    ws = np.asarray(spatial_ker_weights, np.float32)
    wb = np.asarray(bilateral_ker_weights, np.float32)
    cm = np.asarray(compatibility_matrix, np.float32)
    w = np.zeros((CATP, L), np.float32)
    w[0:L] = (cm @ wb).T / FSCALE
    w[32:53] = (cm @ ws).T
    w[64:CATP] = np.eye(L)
    return np.ascontiguousarray(w.astype(np.float16))


def kernel(image, logits, spatial_ker_weights, bilateral_ker_weights,
           compatibility_matrix):
    from concourse.bass_utils import run_bass_kernel_spmd

    if "nc" not in _CACHE:
        _CACHE["nc"] = _build_nc()
    nc = _CACHE["nc"]

    maps = _host_prep(image, logits)
    w12t = _host_w12t(spatial_ker_weights, bilateral_ker_weights,
                      compatibility_matrix)
    for m in maps:
        m["w12t"] = w12t

    res = run_bass_kernel_spmd(nc, maps, core_ids=list(range(NC)))

    out = np.empty((L, D, H, W), dtype=np.float32)
    for r in range(NC):
        blk = res.results[r]["out"]                   # [128, 168]
        out[:, r] = blk.reshape(128, NT, L).transpose(2, 1, 0).reshape(L, H, W)
    return out[None]


# revision 26
# speedup vs baseline: 1.4461x; 1.0170x over previous
"""CRF-RNN (dense Gaussian CRF mean-field) Trainium2 kernel, 8 NeuronCores.

Strategy (v3 — factorized bilateral, latency-tuned)
---------------------------------------------------
N = 8*32*32 = 8192 voxels, L = 21 labels, 5 mean-field iterations.
- Bilateral kernel K_b is rank-36 (quadratic exp fit PC*(PA-d^2/2)^2 as a
  36-dim feature map), so q@K_b = (q@R^T)@C with P = q@R^T a [36,21]
  matrix; K_b is never materialized. Each core ships its own-slice P
  partial inside the per-iteration AllGather; the bilateral message is
  2 matmuls P^T @ featc (1/norm_b and the /FSCALE fold into featc/w12t).
- Spatial kernel is separable: z-mix across the 8 gathered slices, then
  one brt @ (Gy x Gx) pass (fp16, 16 matmuls). The own-slice term is
  prestarted during the AllGather; the other-slice z-mix runs on the PE
  as 8 scaled-identity matmuls (zid input) — the DVE was 3x slower.
- One fp16 AllGather per iteration: [q_own (168) | P partial (21)].
  q-part DMA to the CC buffer is issued right after softmax; the P part
  follows as a tiny second DMA.
- Unary folds into the mixing matmul (cat rows 64:85 = unown^T, w12t
  rows 64:85 = I), so softmax exp reads the mix PSUM directly.
- bil (rows 0:21) and spat (rows 32:53) share one PSUM tile; the cat
  assembly is 2 wide [53,512] copies split across ACT/DVE banks.
- ~18 dummy F=512 matmuls into a scratch PSUM bank after each collective
  keep the PE HAM-warm through the CC window (else every iteration's
  matmuls run at 1.2 GHz).
- Per-core q0/unary blocks are permuted own-slice-first so the SPMD
  program is core-independent; iter-0 P comes from a local 64-matmul
  contraction over the full permuted featrt (no collective needed).
"""

import numpy as np

ALPHA, BETA, GAMMA = 160.0, 3.0, 3.0
NUM_ITER = 5
L, D, H, W = 21, 8, 32, 32
NC = 8
NYX = H * W            # 1024
N = D * NYX            # 8192
NT = NYX // 128        # 8 chunks per slice
FB = NT * L            # 168  free width of one q block
NF = 36                # polynomial feature-map width
FSCALE = 1024.0        # featc pre-scale (folded back via w12t bil rows)
CCW = FB + 24          # 192: 168 q cols + 21 P cols + 3 pad
CATP = 85              # cat rows: 0:21 bil, 32:53 spat, 64:85 unownT


PA = 2.105             # quadratic exp fit: exp(x) ~= PC*(x+PA)^2 on [-0.21,0]
PC = 0.22538087

_CACHE = {}


def _build_nc():
    import concourse.bass as bass  # noqa: F401
    import concourse.bacc as bacc
    import concourse.mybir as mybir
    import concourse.tile as tile
    import concourse.tile_utils as tile_utils

    try:
        tile_utils.max_sbuf_usage = 204 * 1024
    except Exception:
        pass

    f32 = mybir.dt.float32
    f16 = mybir.dt.float16
    AF = mybir.ActivationFunctionType
    OP = mybir.AluOpType

    nc = bacc.Bacc(None, target_bir_lowering=False, num_devices=NC)

    unary_d = nc.declare_dram_parameter("unaryt", [128, NC * FB], f16, isOutput=False)
    featrt_d = nc.declare_dram_parameter("featrt", [128, NC * NT * NF], f16, isOutput=False)
    featc_d = nc.declare_dram_parameter("featc", [NF, NYX], f16, isOutput=False)
    kyx_d = nc.declare_dram_parameter("kyx", [128, NT * NYX], f16, isOutput=False)
    zid_d = nc.declare_dram_parameter("zid", [128, NC * 128], f16, isOutput=False)
    zcor0_d = nc.declare_dram_parameter("zcor0", [128, NC], f32, isOutput=False)
    zcoo_d = nc.declare_dram_parameter("zcoo", [128, 1], f32, isOutput=False)
    w12t_d = nc.declare_dram_parameter("w12t", [CATP, L], f16, isOutput=False)
    unownt_d = nc.declare_dram_parameter("unownt", [L, NYX], f16, isOutput=False)
    out_d = nc.declare_dram_parameter("out", [128, FB], f32, isOutput=True)

    with tile.TileContext(nc) as tc:
        with (
            tc.tile_pool(name="persist", bufs=1) as pp,
            tc.tile_pool(name="work", bufs=4) as wp,
            tc.tile_pool(name="slots", bufs=2) as slp,
            tc.tile_pool(name="stag", bufs=2) as stp,
            tc.tile_pool(name="dram", bufs=2, space="DRAM") as dp,
            tc.tile_pool(name="ps_sp", bufs=1, space="PSUM") as ps_sp,
            tc.tile_pool(name="ps_ct", bufs=1, space="PSUM") as ps_ct_p,
            tc.tile_pool(name="ps_p", bufs=1, space="PSUM") as ps_p,
            tc.tile_pool(name="ps_brt", bufs=1, space="PSUM") as ps_brt_p,
        ):
            sb_unary = pp.tile([128, NC * FB], f16, tag="unary")
            sb_featrt = pp.tile([128, NC * NT * NF], f16, tag="featrt")
            sb_featc = pp.tile([NF, NYX], f16, tag="featc")
            sb_kyx = pp.tile([128, NT * NYX], f16, tag="kyx")
            sb_zid = pp.tile([128, NC * 128], f16, tag="zid")
            sb_zcor0 = pp.tile([128, NC], f32, tag="zcor0")
            sb_zcoo = pp.tile([128, 1], f32, tag="zcoo")
            sb_w12t = pp.tile([CATP, L], f16, tag="w12t")
            sb_unt = pp.tile([L, NYX], f16, tag="unt")
            cats = [pp.tile([CATP, 512], f16, tag=f"cat{h}", name=f"cat{h}")
                    for h in range(2)]
            sb_ex0 = pp.tile([128, NC * FB], f32, tag="ex0")
            sb_red0 = pp.tile([128, NC * NT], f32, tag="red0")
            sb_rcp0 = pp.tile([128, NC * NT], f32, tag="rcp0")
            sb_q0 = pp.tile([128, NC * FB], f16, tag="q0")
            sb_out = pp.tile([128, FB], f32, tag="outt")
            sb_warm = pp.tile([128, 1], f32, tag="warm")
            sb_warm2 = pp.tile([128, 1], f32, tag="warm2")

            nc.sync.dma_start(sb_unary[:, :], unary_d[:, :])
            nc.sync.dma_start(sb_featrt[:, :], featrt_d[:, :])
            nc.sync.dma_start(sb_featc[:, :], featc_d[:, :])
            nc.sync.dma_start(sb_zcor0[:, :], zcor0_d[:, :])
            nc.sync.dma_start(sb_zcoo[:, :], zcoo_d[:, :])
            nc.sync.dma_start(sb_w12t[:, :], w12t_d[:, :])
            nc.sync.dma_start(sb_unt[:, :], unownt_d[:, :])
            nc.sync.dma_start(sb_zid[:, :], zid_d[:, :])
            nc.sync.dma_start(sb_kyx[:, :], kyx_d[:, :])

            # exp table prewarm (~2.7us, overlaps DMA)
            nc.vector.memset(sb_warm[:, :], 0.0)
            nc.scalar.activation(sb_warm2[:, :], sb_warm[:, :], AF.Exp)

            # persistent PSUM: two bil+spat tile PAIRS (ping-pong), one tile
            # per 512-col half so the ACT/DVE cat copies hit independent
            # tiles (banks) and run concurrently. memset once so the
            # never-matmul-written rows 21:32 can't carry NaN junk.
            sp_tiles = [[ps_sp.tile([53, 512], f32, tag=f"spat{j}{hb}",
                                    name=f"spat{j}{hb}") for hb in range(2)]
                        for j in range(2)]
            for j in range(2):
                for hb in range(2):
                    nc.vector.memset(sp_tiles[j][hb][:, :], 0.0)
            # cat gap rows + unownT rows
            for h in range(2):
                nc.vector.memset(cats[h][:, :], 0.0)
            nc.vector.tensor_copy(cats[0][64:CATP, :], sb_unt[:, 0:512])
            nc.scalar.copy(cats[1][64:CATP, :], sb_unt[:, 512:NYX])

            kyx_v = sb_kyx[:, :].rearrange("p (k c) -> p k c", c=NYX)

            # ---------------- iter-0 softmax over full permuted unary ----
            ex0_v = sb_ex0[:, :].rearrange("p (g l) -> p g l", l=L)
            q0_v = sb_q0[:, :].rearrange("p (g l) -> p g l", l=L)
            HW2 = NC * FB // 2
            G2 = NC * NT // 2
            for h in range(2):
                cs = slice(h * HW2, (h + 1) * HW2)
                gs = slice(h * G2, (h + 1) * G2)
                nc.scalar.activation(sb_ex0[:, cs], sb_unary[:, cs], AF.Exp)
                nc.vector.tensor_reduce(
                    sb_red0[:, gs], ex0_v[:, gs, :], mybir.AxisListType.X, OP.add)
                nc.vector.reciprocal(sb_rcp0[:, gs], sb_red0[:, gs])
                nc.vector.tensor_tensor(
                    q0_v[:, gs, :], ex0_v[:, gs, :],
                    sb_rcp0[:, gs].broadcast_to([128, G2, L]), OP.mult)

            # ---------------- iter-0 P over all 64 (permuted) chunks ----
            psP0 = ps_p.tile([NF, L], f32, tag="pp")
            for g in range(NC * NT):
                nc.tensor.matmul(
                    psP0[:, :], sb_featrt[:, g * NF:(g + 1) * NF],
                    sb_q0[:, g * L:(g + 1) * L],
                    start=(g == 0), stop=(g == NC * NT - 1))

            # prestart iter-0 own-slice spatial (overlaps kyx DMA tail)
            bo0 = wp.tile([128, FB], f16, tag="bo")
            nc.vector.tensor_scalar_mul(bo0[:, :], sb_q0[:, 0:FB], sb_zcoo[:, 0:1])
            bo0_v = bo0[:, :].rearrange("p (t l) -> p t l", l=L)
            sp_cur = sp_tiles[0]
            for k in range(NT):
                for hb in range(2):
                    nc.tensor.matmul(
                        sp_cur[hb][32:53, :], bo0_v[:, k, :],
                        kyx_v[:, k, hb * 512:(hb + 1) * 512],
                        start=(k == 0), stop=False, skip_group_check=True)

            # ================= iterations =================
            slots = None
            bo = bo0
            for it in range(NUM_ITER):
                last = it == NUM_ITER - 1

                # ---- z-mix of the other 7 slices -> brt (fp16 SBUF) ----
                brt = wp.tile([128, FB], f16, tag="brt")
                if it == 0:
                    # local q0, DVE path (runs during the first-CC wait)
                    qsrc = sb_q0[:, :].rearrange("p (j f) -> p j f", j=NC)
                    nc.vector.tensor_scalar_mul(
                        brt[:, :], qsrc[:, 1, 0:FB], sb_zcor0[:, 1:2])
                    for d in range(2, NC):
                        nc.vector.scalar_tensor_tensor(
                            brt[:, :], qsrc[:, d, 0:FB], sb_zcor0[:, d:d + 1],
                            brt[:, :], OP.mult, OP.add)
                else:
                    # gathered slots, PE path: 8 scaled-identity matmuls,
                    # pipelined with the 8 per-rank unload DMAs
                    ps_brt = ps_brt_p.tile([128, FB], f32, tag="brtp")
                    for d in range(NC):
                        nc.tensor.matmul(
                            ps_brt[:, :], sb_zid[:, d * 128:(d + 1) * 128],
                            slots[:, d * CCW:d * CCW + FB],
                            start=(d == 0), stop=(d == NC - 1))
                    nc.vector.tensor_copy(brt[:, :], ps_brt[:, :])

                # ---- spatial rest (accumulates onto the prestart) ----
                brt_v = brt[:, :].rearrange("p (t l) -> p t l", l=L)
                for k in range(NT):
                    for hb in range(2):
                        nc.tensor.matmul(
                            sp_cur[hb][32:53, :], brt_v[:, k, :],
                            kyx_v[:, k, hb * 512:(hb + 1) * 512],
                            start=False, stop=(k == NT - 1),
                            skip_group_check=True)

                # ---- P: reduce gathered partials (or copy local P0) ----
                sbP = wp.tile([NF, L], f16, tag="sbp")
                if it == 0:
                    nc.scalar.copy(sbP[:, :], psP0[:, :])
                else:
                    pv = slots[0:NF, :].rearrange("p (d f) -> p d f", d=NC)
                    t1 = wp.tile([NF, 4 * L], f32, tag="pt1")
                    t1v = t1[:, :].rearrange("p (a l) -> p a l", l=L)
                    nc.vector.tensor_tensor(
                        t1v, pv[:, 0:4, FB:FB + L], pv[:, 4:8, FB:FB + L], OP.add)
                    t2 = wp.tile([NF, 2 * L], f32, tag="pt2")
                    t2v = t2[:, :].rearrange("p (a l) -> p a l", l=L)
                    nc.vector.tensor_tensor(t2v, t1v[:, 0:2, :], t1v[:, 2:4, :], OP.add)
                    nc.vector.tensor_tensor(sbP[:, :], t2v[:, 0, :], t2v[:, 1, :], OP.add)

                # ---- bilateral message into rows 0:21 of the same tiles ----
                for hb in range(2):
                    nc.tensor.matmul(
                        sp_cur[hb][0:L, :], sbP[:, :],
                        sb_featc[:, hb * 512:(hb + 1) * 512],
                        start=True, stop=True, skip_group_check=True)

                # ---- cat assembly: 2 wide copies, ACT/DVE on separate tiles
                nc.scalar.copy(cats[0][0:53, :], sp_cur[0][:, :])
                nc.vector.tensor_copy(cats[1][0:53, :], sp_cur[1][:, :])

                # ---- mixing matmul (+unary via identity rows) ----
                psct = ps_ct_p.tile([128, FB], f32, tag="ct")
                for g in range(NT):
                    h, tl = g // 4, g % 4
                    nc.tensor.matmul(
                        psct[:, g * L:(g + 1) * L],
                        cats[h][:, tl * 128:(tl + 1) * 128],
                        sb_w12t[:, :], start=True, stop=True)

                # ---- softmax ----
                ex = wp.tile([128, FB], f32, tag="ex")
                nc.scalar.activation(ex[:, :], psct[:, :], AF.Exp)
                ex_v = ex[:, :].rearrange("p (t l) -> p t l", l=L)
                red = wp.tile([128, NT], f32, tag="red")
                nc.vector.tensor_reduce(red[:, :], ex_v, mybir.AxisListType.X, OP.add)
                rcp = wp.tile([128, NT], f32, tag="rcp")
                nc.vector.reciprocal(rcp[:, :], red[:, :])
                if last:
                    out_v = sb_out[:, :].rearrange("p (t l) -> p t l", l=L)
                    nc.vector.tensor_tensor(
                        out_v, ex_v, rcp[:, :].broadcast_to([128, NT, L]), OP.mult)
                    nc.sync.dma_start(out_d[:, :], sb_out[:, :])
                else:
                    stag = stp.tile([128, CCW], f16, tag="stag")
                    q_v = stag[:, 0:FB].rearrange("p (t l) -> p t l", l=L)
                    nc.vector.tensor_tensor(
                        q_v, ex_v, rcp[:, :].broadcast_to([128, NT, L]), OP.mult)
                    cc_in = dp.tile([128, CCW], f16, tag="ccin")
                    cc_out = dp.tile([128 * NC, CCW], f16, tag="ccout")
                    # ship the q part immediately; P follows as a tiny DMA
                    # launched from the scalar queue (overlapped launches)
                    nc.sync.dma_start(cc_in[:, 0:FB], stag[:, 0:FB])
                    psP = ps_p.tile([NF, L], f32, tag="pp")
                    for t in range(NT):
                        nc.tensor.matmul(
                            psP[:, :], sb_featrt[:, t * NF:(t + 1) * NF],
                            stag[:, t * L:(t + 1) * L],
                            start=(t == 0), stop=(t == NT - 1))
                    nc.scalar.copy(stag[0:NF, FB:FB + L], psP[:, :])
                    nc.scalar.dma_start(cc_in[0:NF, FB:FB + L], stag[0:NF, FB:FB + L])
                    bo = wp.tile([128, FB], f16, tag="bo")
                    nc.vector.tensor_scalar_mul(
                        bo[:, :], stag[:, 0:FB], sb_zcoo[:, 0:1])
                    nc.gpsimd.collective_compute(
                        "AllGather", mybir.AluOpType.bypass,
                        replica_groups=[list(range(NC))],
                        ins=[cc_in.opt()], outs=[cc_out.opt()],
                    )
                    # prestart next iteration's own-slice spatial during CC
                    sp_next = sp_tiles[(it + 1) % 2]
                    bo_v = bo[:, :].rearrange("p (t l) -> p t l", l=L)
                    for k in range(NT):
                        for hb in range(2):
                            nc.tensor.matmul(
                                sp_next[hb][32:53, :],
                                bo_v[:, k, :],
                                kyx_v[:, k, hb * 512:(hb + 1) * 512],
                                start=(k == 0), stop=False,
                                skip_group_check=True)
                    # unload gathered q + P partials: 4 two-rank DMAs whose
                    # launches run on four different sequencers in parallel
                    slots = slp.tile([128, NC * CCW], f16, tag="slots")
                    for d0, dn, eng in ((0, 3, nc.sync), (3, 3, nc.scalar),
                                       (6, 2, nc.gpsimd)):
                        eng.dma_start(
                            slots[:, d0 * CCW:(d0 + dn) * CCW].rearrange(
                                "p (d f) -> p d f", d=dn),
                            cc_out[d0 * 128:(d0 + dn) * 128, :].rearrange(
                                "(d p) f -> p d f", p=128))
                    sp_cur = sp_next
    nc.compile()
    return nc


def _host_prep(image, logits):
    """Per-core input dicts. q0/featrt blocks are permuted own-slice-first."""
    img = np.asarray(image, dtype=np.float32)[0]      # [3, D, H, W]
    lg = np.asarray(logits, dtype=np.float32)[0]      # [L, D, H, W]

    zz, yy, xx = np.meshgrid(
        np.arange(D), np.arange(H), np.arange(W), indexing="ij")
    pos = np.stack([zz, yy, xx], -1).reshape(N, 3).astype(np.float32)
    rgb = img.reshape(3, N).T
    feat = np.concatenate([pos / ALPHA, rgb / BETA], axis=1).astype(np.float16)
    f = feat.astype(np.float32)                       # [N, 6] fp16-rounded
    sq = np.sum(f * f, axis=1)
    al = PA / 2 - sq / 2                              # alpha == beta per formula

    pairs = [(a, b) for a in range(6) for b in range(a + 1, 6)]

    def mono_row(ff, alv):
        cols = [PC * alv * alv, PC * np.ones_like(alv), PC * 2 * alv]
        cols += [PC * 2 * alv * ff[:, a] for a in range(6)]
        cols += [PC * 2 * ff[:, a] for a in range(6)]
        cols += [PC * ff[:, a] ** 2 for a in range(6)]
        cols += [PC * 2 * ff[:, a] * ff[:, b] for a, b in pairs]
        return np.stack(cols, 0)                      # [36, n]

    def mono_col(ff, bev):
        cols = [np.ones_like(bev), bev * bev, bev]
        cols += [ff[:, a] for a in range(6)]
        cols += [bev * ff[:, a] for a in range(6)]
        cols += [ff[:, a] ** 2 for a in range(6)]
        cols += [ff[:, a] * ff[:, b] for a, b in pairs]
        return np.stack(cols, 0)                      # [36, n]

    featr = mono_row(f, al).astype(np.float16)        # [36, N]
    rsum = featr.astype(np.float32).sum(axis=1)       # [36] for the normalizer
    # voxel-major feature chunks: [128, d, t, f]
    featrt_blk = featr.reshape(NF, D, NT, 128).transpose(3, 1, 2, 0)

    r1 = np.arange(D, dtype=np.float32)
    Gz = np.exp(-0.5 * ((r1[:, None] - r1[None, :]) / GAMMA) ** 2)
    r2 = np.arange(H, dtype=np.float32)
    Gy = np.exp(-0.5 * ((r2[:, None] - r2[None, :]) / GAMMA) ** 2)
    Kyx = np.kron(Gy, Gy).astype(np.float32)          # H == W so Gy == Gx
    nyx = Kyx.sum(axis=0)
    Kyx_n = (Kyx / nyx[None, :]).astype(np.float16)
    czsum = Gz.sum(axis=0)
    kyx_in = np.ascontiguousarray(
        Kyx_n.reshape(NT, 128, NYX).transpose(1, 0, 2).reshape(128, NT * NYX))

    unary = lg.reshape(L, N)
    # voxel-major blocks: blkT[p, d, t*L+l] = unary[l, d*NYX + t*128 + p]
    blkT = unary.astype(np.float16).reshape(L, D, NT, 128).transpose(3, 1, 2, 0)

    eye = np.eye(128, dtype=np.float32)
    maps = []
    for r in range(NC):
        sl = slice(r * NYX, (r + 1) * NYX)
        featc = mono_col(f[sl], al[sl]).astype(np.float16)      # [36, 1024]
        norm = rsum @ featc.astype(np.float32)                  # [1024]
        featc_n = (featc.astype(np.float32)
                   * (FSCALE / norm)[None, :]).astype(np.float16)
        perm = [r] + [d for d in range(NC) if d != r]
        un = np.ascontiguousarray(blkT[:, perm].reshape(128, NC * FB))
        featrt = np.ascontiguousarray(
            featrt_blk[:, perm].reshape(128, NC * NT * NF))
        zvec = (Gz[:, r] / czsum[r]).astype(np.float32)
        zcor0 = zvec[perm].copy()
        zcor0[0] = 0.0
        zcor = zvec.copy()
        zcor[r] = 0.0                                 # own slice via prestart
        zid = np.ascontiguousarray(
            (eye[None, :, :] * zcor[:, None, None])
            .transpose(1, 0, 2).reshape(128, NC * 128).astype(np.float16))
        unownt = np.ascontiguousarray(unary[:, sl].astype(np.float16))
        maps.append({
            "unaryt": un,
            "featrt": featrt,
            "featc": np.ascontiguousarray(featc_n),
            "kyx": kyx_in,
            "zid": zid,
            "zcor0": np.ascontiguousarray(np.tile(zcor0, (128, 1))),
            "zcoo": np.full((128, 1), zvec[r], np.float32),
            "unownt": unownt,
        })
    return maps


def _host_w12t(spatial_ker_weights, bilateral_ker_weights, compatibility_matrix):
    ws = np.asarray(spatial_ker_weights, np.float32)
    wb = np.asarray(bilateral_ker_weights, np.float32)
    cm = np.asarray(compatibility_matrix, np.float32)
    w = np.zeros((CATP, L), np.float32)
    w[0:L] = (cm @ wb).T / FSCALE
    w[32:53] = (cm @ ws).T
    w[64:CATP] = np.eye(L)
    return np.ascontiguousarray(w.astype(np.float16))


def kernel(image, logits, spatial_ker_weights, bilateral_ker_weights,
           compatibility_matrix):
    from concourse.bass_utils import run_bass_kernel_spmd

    if "nc" not in _CACHE:
        _CACHE["nc"] = _build_nc()
    nc = _CACHE["nc"]

    maps = _host_prep(image, logits)
    w12t = _host_w12t(spatial_ker_weights, bilateral_ker_weights,
                      compatibility_matrix)
    for m in maps:
        m["w12t"] = w12t

    res = run_bass_kernel_spmd(nc, maps, core_ids=list(range(NC)))

    out = np.empty((L, D, H, W), dtype=np.float32)
    for r in range(NC):
        blk = res.results[r]["out"]                   # [128, 168]
        out[:, r] = blk.reshape(128, NT, L).transpose(2, 1, 0).reshape(L, H, W)
    return out[None]


# revision 28
# speedup vs baseline: 1.4490x; 1.0020x over previous
"""CRF-RNN (dense Gaussian CRF mean-field) Trainium2 kernel, 8 NeuronCores.

Strategy (v3 — factorized bilateral, latency-tuned)
---------------------------------------------------
N = 8*32*32 = 8192 voxels, L = 21 labels, 5 mean-field iterations.
- Bilateral kernel K_b is rank-36 (quadratic exp fit PC*(PA-d^2/2)^2 as a
  36-dim feature map), so q@K_b = (q@R^T)@C with P = q@R^T a [36,21]
  matrix; K_b is never materialized. Each core ships its own-slice P
  partial inside the per-iteration AllGather; the bilateral message is
  2 matmuls P^T @ featc (1/norm_b and the /FSCALE fold into featc/w12t).
- Spatial kernel is separable: z-mix across the 8 gathered slices, then
  one brt @ (Gy x Gx) pass (fp16, 16 matmuls). The own-slice term is
  prestarted during the AllGather; the other-slice z-mix runs on the PE
  as 8 scaled-identity matmuls (zid input) — the DVE was 3x slower.
- One fp16 AllGather per iteration: [q_own (168) | P partial (21)].
  q-part DMA to the CC buffer is issued right after softmax; the P part
  follows as a tiny second DMA.
- Unary folds into the mixing matmul (cat rows 64:85 = unown^T, w12t
  rows 64:85 = I), so softmax exp reads the mix PSUM directly.
- bil (rows 0:21) and spat (rows 32:53) share one PSUM tile; the cat
  assembly is 2 wide [53,512] copies split across ACT/DVE banks.
- ~18 dummy F=512 matmuls into a scratch PSUM bank after each collective
  keep the PE HAM-warm through the CC window (else every iteration's
  matmuls run at 1.2 GHz).
- Per-core q0/unary blocks are permuted own-slice-first so the SPMD
  program is core-independent; iter-0 P comes from a local 64-matmul
  contraction over the full permuted featrt (no collective needed).
"""

import numpy as np

ALPHA, BETA, GAMMA = 160.0, 3.0, 3.0
NUM_ITER = 5
L, D, H, W = 21, 8, 32, 32
NC = 8
NYX = H * W            # 1024
N = D * NYX            # 8192
NT = NYX // 128        # 8 chunks per slice
FB = NT * L            # 168  free width of one q block
NF = 36                # polynomial feature-map width
FSCALE = 1024.0        # featc pre-scale (folded back via w12t bil rows)
CCW = FB + 24          # 192: 168 q cols + 21 P cols + 3 pad
CATP = 85              # cat rows: 0:21 bil, 32:53 spat, 64:85 unownT


PA = 2.105             # quadratic exp fit: exp(x) ~= PC*(x+PA)^2 on [-0.21,0]
PC = 0.22538087

_CACHE = {}


def _build_nc():
    import concourse.bass as bass  # noqa: F401
    import concourse.bacc as bacc
    import concourse.mybir as mybir
    import concourse.tile as tile
    import concourse.tile_utils as tile_utils

    try:
        tile_utils.max_sbuf_usage = 204 * 1024
    except Exception:
        pass

    f32 = mybir.dt.float32
    f16 = mybir.dt.float16
    AF = mybir.ActivationFunctionType
    OP = mybir.AluOpType

    nc = bacc.Bacc(None, target_bir_lowering=False, num_devices=NC)

    unary_d = nc.declare_dram_parameter("unaryt", [128, NC * FB], f16, isOutput=False)
    featrt_d = nc.declare_dram_parameter("featrt", [128, NC * NT * NF], f16, isOutput=False)
    featc_d = nc.declare_dram_parameter("featc", [NF, NYX], f16, isOutput=False)
    kyx_d = nc.declare_dram_parameter("kyx", [128, NT * NYX], f16, isOutput=False)
    zid_d = nc.declare_dram_parameter("zid", [128, NC * 128], f16, isOutput=False)
    zcor0_d = nc.declare_dram_parameter("zcor0", [128, NC], f32, isOutput=False)
    zcoo_d = nc.declare_dram_parameter("zcoo", [128, 1], f32, isOutput=False)
    w12t_d = nc.declare_dram_parameter("w12t", [CATP, L], f16, isOutput=False)
    unownt_d = nc.declare_dram_parameter("unownt", [L, NYX], f16, isOutput=False)
    out_d = nc.declare_dram_parameter("out", [128, FB], f32, isOutput=True)

    with tile.TileContext(nc) as tc:
        with (
            tc.tile_pool(name="persist", bufs=1) as pp,
            tc.tile_pool(name="work", bufs=4) as wp,
            tc.tile_pool(name="slots", bufs=2) as slp,
            tc.tile_pool(name="stag", bufs=2) as stp,
            tc.tile_pool(name="dram", bufs=2, space="DRAM") as dp,
            tc.tile_pool(name="ps_sp", bufs=1, space="PSUM") as ps_sp,
            tc.tile_pool(name="ps_ct", bufs=1, space="PSUM") as ps_ct_p,
            tc.tile_pool(name="ps_p", bufs=1, space="PSUM") as ps_p,
            tc.tile_pool(name="ps_brt", bufs=1, space="PSUM") as ps_brt_p,
        ):
            sb_wu = pp.tile([128, 8], f16, tag="wu")
            sb_unary = pp.tile([128, NC * FB], f16, tag="unary")
            sb_featrt = pp.tile([128, NC * NT * NF], f16, tag="featrt")
            sb_featc = pp.tile([NF, NYX], f16, tag="featc")
            sb_kyx = pp.tile([128, NT * NYX], f16, tag="kyx")
            sb_zid = pp.tile([128, NC * 128], f16, tag="zid")
            sb_zcor0 = pp.tile([128, NC], f32, tag="zcor0")
            sb_zcoo = pp.tile([128, 1], f32, tag="zcoo")
            sb_w12t = pp.tile([CATP, L], f16, tag="w12t")
            sb_unt = pp.tile([L, NYX], f16, tag="unt")
            cats = [pp.tile([CATP, 512], f16, tag=f"cat{h}", name=f"cat{h}")
                    for h in range(2)]
            sb_ex0 = pp.tile([128, NC * FB], f32, tag="ex0")
            sb_red0 = pp.tile([128, NC * NT], f32, tag="red0")
            sb_rcp0 = pp.tile([128, NC * NT], f32, tag="rcp0")
            sb_q0 = pp.tile([128, NC * FB], f16, tag="q0")
            sb_out = pp.tile([128, FB], f32, tag="outt")
            sb_warm = pp.tile([128, 1], f32, tag="warm")
            sb_warm2 = pp.tile([128, 1], f32, tag="warm2")

            # collective warmup FIRST: ncfw init (~50us) runs from the FIRST
            # doorbell, so fire one ASAP — its DMA heads the sync queue.
            nc.vector.memset(sb_wu[:, :], 0.0)
            wu_in = dp.tile([128, 8], f16, tag="wuin")
            wu_out = dp.tile([128 * NC, 8], f16, tag="wuout")
            nc.sync.dma_start(wu_in[:, :], sb_wu[:, :])
            nc.gpsimd.collective_compute(
                "AllGather", mybir.AluOpType.bypass,
                replica_groups=[list(range(NC))],
                ins=[wu_in.opt()], outs=[wu_out.opt()],
            )

            nc.sync.dma_start(sb_unary[:, :], unary_d[:, :])
            nc.sync.dma_start(sb_featrt[:, :], featrt_d[:, :])
            nc.sync.dma_start(sb_featc[:, :], featc_d[:, :])
            nc.sync.dma_start(sb_zcor0[:, :], zcor0_d[:, :])
            nc.sync.dma_start(sb_zcoo[:, :], zcoo_d[:, :])
            nc.sync.dma_start(sb_w12t[:, :], w12t_d[:, :])
            nc.sync.dma_start(sb_unt[:, :], unownt_d[:, :])
            nc.sync.dma_start(sb_zid[:, :], zid_d[:, :])
            nc.sync.dma_start(sb_kyx[:, :], kyx_d[:, :])

            # exp table prewarm (~2.7us, overlaps DMA)
            nc.vector.memset(sb_warm[:, :], 0.0)
            nc.scalar.activation(sb_warm2[:, :], sb_warm[:, :], AF.Exp)

            # persistent PSUM: two bil+spat tile PAIRS (ping-pong), one tile
            # per 512-col half so the ACT/DVE cat copies hit independent
            # tiles (banks) and run concurrently. memset once so the
            # never-matmul-written rows 21:32 can't carry NaN junk.
            sp_tiles = [[ps_sp.tile([53, 512], f32, tag=f"spat{j}{hb}",
                                    name=f"spat{j}{hb}") for hb in range(2)]
                        for j in range(2)]
            for j in range(2):
                for hb in range(2):
                    nc.vector.memset(sp_tiles[j][hb][:, :], 0.0)
            # cat gap rows + unownT rows
            for h in range(2):
                nc.vector.memset(cats[h][:, :], 0.0)
            nc.vector.tensor_copy(cats[0][64:CATP, :], sb_unt[:, 0:512])
            nc.scalar.copy(cats[1][64:CATP, :], sb_unt[:, 512:NYX])

            kyx_v = sb_kyx[:, :].rearrange("p (k c) -> p k c", c=NYX)

            # ---------------- iter-0 softmax over full permuted unary ----
            ex0_v = sb_ex0[:, :].rearrange("p (g l) -> p g l", l=L)
            q0_v = sb_q0[:, :].rearrange("p (g l) -> p g l", l=L)
            HW2 = NC * FB // 2
            G2 = NC * NT // 2
            for h in range(2):
                cs = slice(h * HW2, (h + 1) * HW2)
                gs = slice(h * G2, (h + 1) * G2)
                nc.scalar.activation(sb_ex0[:, cs], sb_unary[:, cs], AF.Exp)
                nc.vector.tensor_reduce(
                    sb_red0[:, gs], ex0_v[:, gs, :], mybir.AxisListType.X, OP.add)
                nc.vector.reciprocal(sb_rcp0[:, gs], sb_red0[:, gs])
                nc.vector.tensor_tensor(
                    q0_v[:, gs, :], ex0_v[:, gs, :],
                    sb_rcp0[:, gs].broadcast_to([128, G2, L]), OP.mult)

            # ---------------- iter-0 P over all 64 (permuted) chunks ----
            psP0 = ps_p.tile([NF, L], f32, tag="pp")
            for g in range(NC * NT):
                nc.tensor.matmul(
                    psP0[:, :], sb_featrt[:, g * NF:(g + 1) * NF],
                    sb_q0[:, g * L:(g + 1) * L],
                    start=(g == 0), stop=(g == NC * NT - 1))

            # prestart iter-0 own-slice spatial (overlaps kyx DMA tail)
            bo0 = wp.tile([128, FB], f16, tag="bo")
            nc.vector.tensor_scalar_mul(bo0[:, :], sb_q0[:, 0:FB], sb_zcoo[:, 0:1])
            bo0_v = bo0[:, :].rearrange("p (t l) -> p t l", l=L)
            sp_cur = sp_tiles[0]
            for k in range(NT):
                for hb in range(2):
                    nc.tensor.matmul(
                        sp_cur[hb][32:53, :], bo0_v[:, k, :],
                        kyx_v[:, k, hb * 512:(hb + 1) * 512],
                        start=(k == 0), stop=False, skip_group_check=True)

            # ================= iterations =================
            slots = None
            bo = bo0
            for it in range(NUM_ITER):
                last = it == NUM_ITER - 1

                # ---- z-mix of the other 7 slices -> brt (fp16 SBUF) ----
                brt = wp.tile([128, FB], f16, tag="brt")
                if it == 0:
                    # local q0, DVE path (runs during the first-CC wait)
                    qsrc = sb_q0[:, :].rearrange("p (j f) -> p j f", j=NC)
                    nc.vector.tensor_scalar_mul(
                        brt[:, :], qsrc[:, 1, 0:FB], sb_zcor0[:, 1:2])
                    for d in range(2, NC):
                        nc.vector.scalar_tensor_tensor(
                            brt[:, :], qsrc[:, d, 0:FB], sb_zcor0[:, d:d + 1],
                            brt[:, :], OP.mult, OP.add)
                else:
                    # gathered slots, PE path: 8 scaled-identity matmuls,
                    # pipelined with the 8 per-rank unload DMAs
                    ps_brt = ps_brt_p.tile([128, FB], f32, tag="brtp")
                    for d in range(NC):
                        nc.tensor.matmul(
                            ps_brt[:, :], sb_zid[:, d * 128:(d + 1) * 128],
                            slots[:, d * CCW:d * CCW + FB],
                            start=(d == 0), stop=(d == NC - 1))
                    nc.vector.tensor_copy(brt[:, :], ps_brt[:, :])

                # ---- spatial rest (accumulates onto the prestart) ----
                brt_v = brt[:, :].rearrange("p (t l) -> p t l", l=L)
                for k in range(NT):
                    for hb in range(2):
                        nc.tensor.matmul(
                            sp_cur[hb][32:53, :], brt_v[:, k, :],
                            kyx_v[:, k, hb * 512:(hb + 1) * 512],
                            start=False, stop=(k == NT - 1),
                            skip_group_check=True)

                # ---- P: reduce gathered partials (or copy local P0) ----
                sbP = wp.tile([NF, L], f16, tag="sbp")
                if it == 0:
                    nc.scalar.copy(sbP[:, :], psP0[:, :])
                else:
                    pv = slots[0:NF, :].rearrange("p (d f) -> p d f", d=NC)
                    t1 = wp.tile([NF, 4 * L], f32, tag="pt1")
                    t1v = t1[:, :].rearrange("p (a l) -> p a l", l=L)
                    nc.vector.tensor_tensor(
                        t1v, pv[:, 0:4, FB:FB + L], pv[:, 4:8, FB:FB + L], OP.add)
                    t2 = wp.tile([NF, 2 * L], f32, tag="pt2")
                    t2v = t2[:, :].rearrange("p (a l) -> p a l", l=L)
                    nc.vector.tensor_tensor(t2v, t1v[:, 0:2, :], t1v[:, 2:4, :], OP.add)
                    nc.vector.tensor_tensor(sbP[:, :], t2v[:, 0, :], t2v[:, 1, :], OP.add)

                # ---- bilateral message into rows 0:21 of the same tiles ----
                for hb in range(2):
                    nc.tensor.matmul(
                        sp_cur[hb][0:L, :], sbP[:, :],
                        sb_featc[:, hb * 512:(hb + 1) * 512],
                        start=True, stop=True, skip_group_check=True)

                # ---- cat assembly: 2 wide copies, ACT/DVE on separate tiles
                nc.scalar.copy(cats[0][0:53, :], sp_cur[0][:, :])
                nc.vector.tensor_copy(cats[1][0:53, :], sp_cur[1][:, :])

                # ---- mixing matmul (+unary via identity rows) ----
                psct = ps_ct_p.tile([128, FB], f32, tag="ct")
                for g in range(NT):
                    h, tl = g // 4, g % 4
                    nc.tensor.matmul(
                        psct[:, g * L:(g + 1) * L],
                        cats[h][:, tl * 128:(tl + 1) * 128],
                        sb_w12t[:, :], start=True, stop=True)

                # ---- softmax ----
                ex = wp.tile([128, FB], f32, tag="ex")
                nc.scalar.activation(ex[:, :], psct[:, :], AF.Exp)
                ex_v = ex[:, :].rearrange("p (t l) -> p t l", l=L)
                red = wp.tile([128, NT], f32, tag="red")
                nc.vector.tensor_reduce(red[:, :], ex_v, mybir.AxisListType.X, OP.add)
                rcp = wp.tile([128, NT], f32, tag="rcp")
                nc.vector.reciprocal(rcp[:, :], red[:, :])
                if last:
                    out_v = sb_out[:, :].rearrange("p (t l) -> p t l", l=L)
                    nc.vector.tensor_tensor(
                        out_v, ex_v, rcp[:, :].broadcast_to([128, NT, L]), OP.mult)
                    nc.sync.dma_start(out_d[:, :], sb_out[:, :])
                else:
                    stag = stp.tile([128, CCW], f16, tag="stag")
                    q_v = stag[:, 0:FB].rearrange("p (t l) -> p t l", l=L)
                    nc.vector.tensor_tensor(
                        q_v, ex_v, rcp[:, :].broadcast_to([128, NT, L]), OP.mult)
                    cc_in = dp.tile([128, CCW], f16, tag="ccin")
                    cc_out = dp.tile([128 * NC, CCW], f16, tag="ccout")
                    # ship the q part immediately; P follows as a tiny DMA
                    # launched from the scalar queue (overlapped launches)
                    nc.sync.dma_start(cc_in[:, 0:FB], stag[:, 0:FB])
                    psP = ps_p.tile([NF, L], f32, tag="pp")
                    for t in range(NT):
                        nc.tensor.matmul(
                            psP[:, :], sb_featrt[:, t * NF:(t + 1) * NF],
                            stag[:, t * L:(t + 1) * L],
                            start=(t == 0), stop=(t == NT - 1))
                    nc.scalar.copy(stag[0:NF, FB:FB + L], psP[:, :])
                    nc.scalar.dma_start(cc_in[0:NF, FB:FB + L], stag[0:NF, FB:FB + L])
                    bo = wp.tile([128, FB], f16, tag="bo")
                    nc.vector.tensor_scalar_mul(
                        bo[:, :], stag[:, 0:FB], sb_zcoo[:, 0:1])
                    nc.gpsimd.collective_compute(
                        "AllGather", mybir.AluOpType.bypass,
                        replica_groups=[list(range(NC))],
                        ins=[cc_in.opt()], outs=[cc_out.opt()],
                    )
                    # prestart next iteration's own-slice spatial during CC
                    sp_next = sp_tiles[(it + 1) % 2]
                    bo_v = bo[:, :].rearrange("p (t l) -> p t l", l=L)
                    for k in range(NT):
                        for hb in range(2):
                            nc.tensor.matmul(
                                sp_next[hb][32:53, :],
                                bo_v[:, k, :],
                                kyx_v[:, k, hb * 512:(hb + 1) * 512],
                                start=(k == 0), stop=False,
                                skip_group_check=True)
                    # unload gathered q + P partials: 4 two-rank DMAs whose
                    # launches run on four different sequencers in parallel
                    slots = slp.tile([128, NC * CCW], f16, tag="slots")
                    for d0, dn, eng in ((0, 3, nc.sync), (3, 3, nc.scalar),
                                       (6, 2, nc.gpsimd)):
                        eng.dma_start(
                            slots[:, d0 * CCW:(d0 + dn) * CCW].rearrange(
                                "p (d f) -> p d f", d=dn),
                            cc_out[d0 * 128:(d0 + dn) * 128, :].rearrange(
                                "(d p) f -> p d f", p=128))
                    sp_cur = sp_next
    nc.compile()
    return nc


def _host_prep(image, logits):
    """Per-core input dicts. q0/featrt blocks are permuted own-slice-first."""
    img = np.asarray(image, dtype=np.float32)[0]      # [3, D, H, W]
    lg = np.asarray(logits, dtype=np.float32)[0]      # [L, D, H, W]

    zz, yy, xx = np.meshgrid(
        np.arange(D), np.arange(H), np.arange(W), indexing="ij")
    pos = np.stack([zz, yy, xx], -1).reshape(N, 3).astype(np.float32)
    rgb = img.reshape(3, N).T
    feat = np.concatenate([pos / ALPHA, rgb / BETA], axis=1).astype(np.float16)
    f = feat.astype(np.float32)                       # [N, 6] fp16-rounded
    sq = np.sum(f * f, axis=1)
    al = PA / 2 - sq / 2                              # alpha == beta per formula

    pairs = [(a, b) for a in range(6) for b in range(a + 1, 6)]

    def mono_row(ff, alv):
        cols = [PC * alv * alv, PC * np.ones_like(alv), PC * 2 * alv]
        cols += [PC * 2 * alv * ff[:, a] for a in range(6)]
        cols += [PC * 2 * ff[:, a] for a in range(6)]
        cols += [PC * ff[:, a] ** 2 for a in range(6)]
        cols += [PC * 2 * ff[:, a] * ff[:, b] for a, b in pairs]
        return np.stack(cols, 0)                      # [36, n]

    def mono_col(ff, bev):
        cols = [np.ones_like(bev), bev * bev, bev]
        cols += [ff[:, a] for a in range(6)]
        cols += [bev * ff[:, a] for a in range(6)]
        cols += [ff[:, a] ** 2 for a in range(6)]
        cols += [ff[:, a] * ff[:, b] for a, b in pairs]
        return np.stack(cols, 0)                      # [36, n]

    featr = mono_row(f, al).astype(np.float16)        # [36, N]
    rsum = featr.astype(np.float32).sum(axis=1)       # [36] for the normalizer
    # voxel-major feature chunks: [128, d, t, f]
    featrt_blk = featr.reshape(NF, D, NT, 128).transpose(3, 1, 2, 0)

    r1 = np.arange(D, dtype=np.float32)
    Gz = np.exp(-0.5 * ((r1[:, None] - r1[None, :]) / GAMMA) ** 2)
    r2 = np.arange(H, dtype=np.float32)
    Gy = np.exp(-0.5 * ((r2[:, None] - r2[None, :]) / GAMMA) ** 2)
    Kyx = np.kron(Gy, Gy).astype(np.float32)          # H == W so Gy == Gx
    nyx = Kyx.sum(axis=0)
    Kyx_n = (Kyx / nyx[None, :]).astype(np.float16)
    czsum = Gz.sum(axis=0)
    kyx_in = np.ascontiguousarray(
        Kyx_n.reshape(NT, 128, NYX).transpose(1, 0, 2).reshape(128, NT * NYX))

    unary = lg.reshape(L, N)
    # voxel-major blocks: blkT[p, d, t*L+l] = unary[l, d*NYX + t*128 + p]
    blkT = unary.astype(np.float16).reshape(L, D, NT, 128).transpose(3, 1, 2, 0)

    eye = np.eye(128, dtype=np.float32)
    maps = []
    for r in range(NC):
        sl = slice(r * NYX, (r + 1) * NYX)
        featc = mono_col(f[sl], al[sl]).astype(np.float16)      # [36, 1024]
        norm = rsum @ featc.astype(np.float32)                  # [1024]
        featc_n = (featc.astype(np.float32)
                   * (FSCALE / norm)[None, :]).astype(np.float16)
        perm = [r] + [d for d in range(NC) if d != r]
        un = np.ascontiguousarray(blkT[:, perm].reshape(128, NC * FB))
        featrt = np.ascontiguousarray(
            featrt_blk[:, perm].reshape(128, NC * NT * NF))
        zvec = (Gz[:, r] / czsum[r]).astype(np.float32)
        zcor0 = zvec[perm].copy()
        zcor0[0] = 0.0
        zcor = zvec.copy()
        zcor[r] = 0.0                                 # own slice via prestart
        zid = np.ascontiguousarray(
            (eye[None, :, :] * zcor[:, None, None])
            .transpose(1, 0, 2).reshape(128, NC * 128).astype(np.float16))
        unownt = np.ascontiguousarray(unary[:, sl].astype(np.float16))
        maps.append({
            "unaryt": un,
            "featrt": featrt,
            "featc": np.ascontiguousarray(featc_n),
            "kyx": kyx_in,
            "zid": zid,
            "zcor0": np.ascontiguousarray(np.tile(zcor0, (128, 1))),
            "zcoo": np.full((128, 1), zvec[r], np.float32),
            "unownt": unownt,
        })
    return maps


def _host_w12t(spatial_ker_weights, bilateral_ker_weights, compatibility_matrix):
    ws = np.asarray(spatial_ker_weights, np.float32)
    wb = np.asarray(bilateral_ker_weights, np.float32)
    cm = np.asarray(compatibility_matrix, np.float32)
    w = np.zeros((CATP, L), np.float32)
    w[0:L] = (cm @ wb).T / FSCALE
    w[32:53] = (cm @ ws).T
    w[64:CATP] = np.eye(L)
    return np.ascontiguousarray(w.astype(np.float16))


def kernel(image, logits, spatial_ker_weights, bilateral_ker_weights,
           compatibility_matrix):
    from concourse.bass_utils import run_bass_kernel_spmd

    if "nc" not in _CACHE:
        _CACHE["nc"] = _build_nc()
    nc = _CACHE["nc"]

    maps = _host_prep(image, logits)
    w12t = _host_w12t(spatial_ker_weights, bilateral_ker_weights,
                      compatibility_matrix)
    for m in maps:
        m["w12t"] = w12t

    res = run_bass_kernel_spmd(nc, maps, core_ids=list(range(NC)))

    out = np.empty((L, D, H, W), dtype=np.float32)
    for r in range(NC):
        blk = res.results[r]["out"]                   # [128, 168]
        out[:, r] = blk.reshape(128, NT, L).transpose(2, 1, 0).reshape(L, H, W)
    return out[None]


# revision 34
# speedup vs baseline: 1.5322x; 1.0574x over previous
"""CRF-RNN (dense Gaussian CRF mean-field) Trainium2 kernel, 8 NeuronCores.

Strategy (v3 — factorized bilateral, latency-tuned)
---------------------------------------------------
N = 8*32*32 = 8192 voxels, L = 21 labels, 5 mean-field iterations.
- Bilateral kernel K_b is rank-36 (quadratic exp fit PC*(PA-d^2/2)^2 as a
  36-dim feature map), so q@K_b = (q@R^T)@C with P = q@R^T a [36,21]
  matrix; K_b is never materialized. Each core ships its own-slice P
  partial inside the per-iteration AllGather; the bilateral message is
  2 matmuls P^T @ featc (1/norm_b and the /FSCALE fold into featc/w12t).
- Spatial kernel is separable: z-mix across the 8 gathered slices, then
  one brt @ (Gy x Gx) pass (fp16, 16 matmuls). The own-slice term is
  prestarted during the AllGather; the other-slice z-mix runs on the PE
  as 8 scaled-identity matmuls (zid input) — the DVE was 3x slower.
- One fp16 AllGather per iteration: [q_own (168) | P partial (21)].
  q-part DMA to the CC buffer is issued right after softmax; the P part
  follows as a tiny second DMA.
- Unary folds into the mixing matmul (cat rows 64:85 = unown^T, w12t
  rows 64:85 = I), so softmax exp reads the mix PSUM directly.
- bil (rows 0:21) and spat (rows 32:53) share one PSUM tile; the cat
  assembly is 2 wide [53,512] copies split across ACT/DVE banks.
- ~18 dummy F=512 matmuls into a scratch PSUM bank after each collective
  keep the PE HAM-warm through the CC window (else every iteration's
  matmuls run at 1.2 GHz).
- Per-core q0/unary blocks are permuted own-slice-first so the SPMD
  program is core-independent; iter-0 P comes from a local 64-matmul
  contraction over the full permuted featrt (no collective needed).
"""

import numpy as np

ALPHA, BETA, GAMMA = 160.0, 3.0, 3.0
NUM_ITER = 5
L, D, H, W = 21, 8, 32, 32
NC = 8
NYX = H * W            # 1024
N = D * NYX            # 8192
NT = NYX // 128        # 8 chunks per slice
FB = NT * L            # 168  free width of one q block
NF = 36                # polynomial feature-map width
FSCALE = 1024.0        # featc pre-scale (folded back via w12t bil rows)
CCW = FB + 48          # 216 fp8 bytes: 168 q + 42 P (fp16 bitcast) + 6 pad
CATP = 85              # cat rows: 0:21 bil, 32:53 spat, 64:85 unownT


PA = 2.105             # quadratic exp fit: exp(x) ~= PC*(x+PA)^2 on [-0.21,0]
PC = 0.22538087

_CACHE = {}


def _build_nc():
    import concourse.bass as bass  # noqa: F401
    import concourse.bacc as bacc
    import concourse.mybir as mybir
    import concourse.tile as tile
    import concourse.tile_utils as tile_utils

    try:
        tile_utils.max_sbuf_usage = 204 * 1024
    except Exception:
        pass

    f32 = mybir.dt.float32
    f16 = mybir.dt.float16
    f8 = mybir.dt.float8e4
    AF = mybir.ActivationFunctionType
    OP = mybir.AluOpType

    nc = bacc.Bacc(None, target_bir_lowering=False, num_devices=NC)

    unary_d = nc.declare_dram_parameter("unaryt", [128, NC * FB], f16, isOutput=False)
    featrt_d = nc.declare_dram_parameter("featrt", [128, NC * NT * NF], f16, isOutput=False)
    featc_d = nc.declare_dram_parameter("featc", [NF, NYX], f16, isOutput=False)
    kyx_d = nc.declare_dram_parameter("kyx", [128, NT * NYX], f16, isOutput=False)
    zid_d = nc.declare_dram_parameter("zid", [128, NC * 128], f16, isOutput=False)
    zcor0_d = nc.declare_dram_parameter("zcor0", [128, NC], f32, isOutput=False)
    zcoo_d = nc.declare_dram_parameter("zcoo", [128, 1], f32, isOutput=False)
    w12t_d = nc.declare_dram_parameter("w12t", [CATP, L], f16, isOutput=False)
    unownt_d = nc.declare_dram_parameter("unownt", [L, NYX], f16, isOutput=False)
    out_d = nc.declare_dram_parameter("out", [128, FB], f32, isOutput=True)

    with tile.TileContext(nc) as tc:
        with (
            tc.tile_pool(name="persist", bufs=1) as pp,
            tc.tile_pool(name="work", bufs=4) as wp,
            tc.tile_pool(name="slots", bufs=2) as slp,
            tc.tile_pool(name="stag", bufs=2) as stp,
            tc.tile_pool(name="dram", bufs=2, space="DRAM") as dp,
            tc.tile_pool(name="ps_sp", bufs=1, space="PSUM") as ps_sp,
            tc.tile_pool(name="ps_ct", bufs=1, space="PSUM") as ps_ct_p,
            tc.tile_pool(name="ps_p", bufs=1, space="PSUM") as ps_p,
            tc.tile_pool(name="ps_brt", bufs=1, space="PSUM") as ps_brt_p,
        ):
            sb_wu = pp.tile([128, 8], f16, tag="wu")
            sb_unary = pp.tile([128, NC * FB], f16, tag="unary")
            sb_featrt = pp.tile([128, NC * NT * NF], f16, tag="featrt")
            sb_featc = pp.tile([NF, NYX], f16, tag="featc")
            sb_kyx = pp.tile([128, NT * NYX], f16, tag="kyx")
            sb_zid = pp.tile([128, NC * 128], f16, tag="zid")
            sb_zcor0 = pp.tile([128, NC], f32, tag="zcor0")
            sb_zcoo = pp.tile([128, 1], f32, tag="zcoo")
            sb_w12t = pp.tile([CATP, L], f16, tag="w12t")
            sb_unt = pp.tile([L, NYX], f16, tag="unt")
            cats = [pp.tile([CATP, 512], f16, tag=f"cat{h}", name=f"cat{h}")
                    for h in range(2)]
            sb_ex0 = pp.tile([128, NC * FB], f32, tag="ex0")
            sb_red0 = pp.tile([128, NC * NT], f32, tag="red0")
            sb_rcp0 = pp.tile([128, NC * NT], f32, tag="rcp0")
            sb_q0 = pp.tile([128, NC * FB], f16, tag="q0")
            sb_out = pp.tile([128, FB], f32, tag="outt")
            sb_warm = pp.tile([128, 1], f32, tag="warm")
            sb_warm2 = pp.tile([128, 1], f32, tag="warm2")

            # collective warmup FIRST: ncfw init (~50us) runs from the FIRST
            # doorbell, so fire one ASAP — its DMA heads the sync queue.
            nc.vector.memset(sb_wu[:, :], 0.0)
            wu_in = dp.tile([128, 8], f16, tag="wuin")
            wu_out = dp.tile([128 * NC, 8], f16, tag="wuout")
            nc.sync.dma_start(wu_in[:, :], sb_wu[:, :])
            nc.gpsimd.collective_compute(
                "AllGather", mybir.AluOpType.bypass,
                replica_groups=[list(range(NC))],
                ins=[wu_in.opt()], outs=[wu_out.opt()],
            )

            nc.sync.dma_start(sb_unary[:, :], unary_d[:, :])
            nc.sync.dma_start(sb_featrt[:, :], featrt_d[:, :])
            nc.sync.dma_start(sb_featc[:, :], featc_d[:, :])
            nc.sync.dma_start(sb_zcor0[:, :], zcor0_d[:, :])
            nc.sync.dma_start(sb_zcoo[:, :], zcoo_d[:, :])
            nc.sync.dma_start(sb_w12t[:, :], w12t_d[:, :])
            nc.sync.dma_start(sb_unt[:, :], unownt_d[:, :])
            nc.sync.dma_start(sb_zid[:, :], zid_d[:, :])
            nc.sync.dma_start(sb_kyx[:, :], kyx_d[:, :])

            # exp table prewarm (~2.7us, overlaps DMA)
            nc.vector.memset(sb_warm[:, :], 0.0)
            nc.scalar.activation(sb_warm2[:, :], sb_warm[:, :], AF.Exp)

            # persistent PSUM: two bil+spat tile PAIRS (ping-pong), one tile
            # per 512-col half so the ACT/DVE cat copies hit independent
            # tiles (banks) and run concurrently. memset once so the
            # never-matmul-written rows 21:32 can't carry NaN junk.
            sp_tiles = [[ps_sp.tile([53, 512], f32, tag=f"spat{j}{hb}",
                                    name=f"spat{j}{hb}") for hb in range(2)]
                        for j in range(2)]
            for j in range(2):
                for hb in range(2):
                    nc.vector.memset(sp_tiles[j][hb][:, :], 0.0)
            # cat gap rows + unownT rows
            for h in range(2):
                nc.vector.memset(cats[h][:, :], 0.0)
            nc.vector.tensor_copy(cats[0][64:CATP, :], sb_unt[:, 0:512])
            nc.scalar.copy(cats[1][64:CATP, :], sb_unt[:, 512:NYX])

            kyx_v = sb_kyx[:, :].rearrange("p (k c) -> p k c", c=NYX)

            # ---------------- iter-0 softmax over full permuted unary ----
            ex0_v = sb_ex0[:, :].rearrange("p (g l) -> p g l", l=L)
            q0_v = sb_q0[:, :].rearrange("p (g l) -> p g l", l=L)
            HW2 = NC * FB // 2
            G2 = NC * NT // 2
            for h in range(2):
                cs = slice(h * HW2, (h + 1) * HW2)
                gs = slice(h * G2, (h + 1) * G2)
                nc.scalar.activation(sb_ex0[:, cs], sb_unary[:, cs], AF.Exp)
                nc.vector.tensor_reduce(
                    sb_red0[:, gs], ex0_v[:, gs, :], mybir.AxisListType.X, OP.add)
                nc.vector.reciprocal(sb_rcp0[:, gs], sb_red0[:, gs])
                nc.vector.tensor_tensor(
                    q0_v[:, gs, :], ex0_v[:, gs, :],
                    sb_rcp0[:, gs].broadcast_to([128, G2, L]), OP.mult)

            # ---------------- iter-0 P over all 64 (permuted) chunks ----
            psP0 = ps_p.tile([NF, L], f32, tag="pp")
            for g in range(NC * NT):
                nc.tensor.matmul(
                    psP0[:, :], sb_featrt[:, g * NF:(g + 1) * NF],
                    sb_q0[:, g * L:(g + 1) * L],
                    start=(g == 0), stop=(g == NC * NT - 1))

            # prestart iter-0 own-slice spatial (overlaps kyx DMA tail)
            bo0 = wp.tile([128, FB], f16, tag="bo")
            nc.vector.tensor_scalar_mul(bo0[:, :], sb_q0[:, 0:FB], sb_zcoo[:, 0:1])
            bo0_v = bo0[:, :].rearrange("p (t l) -> p t l", l=L)
            sp_cur = sp_tiles[0]
            for k in range(NT):
                for hb in range(2):
                    nc.tensor.matmul(
                        sp_cur[hb][32:53, :], bo0_v[:, k, :],
                        kyx_v[:, k, hb * 512:(hb + 1) * 512],
                        start=(k == 0), stop=False, skip_group_check=True)

            # ================= iterations =================
            slots = None
            bo = bo0
            for it in range(NUM_ITER):
                last = it == NUM_ITER - 1

                # ---- z-mix of the other 7 slices -> brt (fp16 SBUF) ----
                brt = wp.tile([128, FB], f16, tag="brt")
                if it == 0:
                    # local q0, DVE path (runs during the first-CC wait)
                    qsrc = sb_q0[:, :].rearrange("p (j f) -> p j f", j=NC)
                    nc.vector.tensor_scalar_mul(
                        brt[:, :], qsrc[:, 1, 0:FB], sb_zcor0[:, 1:2])
                    for d in range(2, NC):
                        nc.vector.scalar_tensor_tensor(
                            brt[:, :], qsrc[:, d, 0:FB], sb_zcor0[:, d:d + 1],
                            brt[:, :], OP.mult, OP.add)
                else:
                    # gathered fp8 slots -> fp16 (DVE, pipelined per unload
                    # group), then PE path: 8 scaled-identity matmuls
                    q16g = wp.tile([128, NC * FB], f16, tag="q16g")
                    for d0, dn in ((0, 3), (3, 3), (6, 2)):
                        nc.vector.tensor_copy(
                            q16g[:, d0 * FB:(d0 + dn) * FB].rearrange(
                                "p (d f) -> p d f", d=dn),
                            slots[:, :].rearrange(
                                "p (d f) -> p d f", d=NC)[:, d0:d0 + dn, 0:FB])
                    ps_brt = ps_brt_p.tile([128, FB], f32, tag="brtp")
                    for d in range(NC):
                        nc.tensor.matmul(
                            ps_brt[:, :], sb_zid[:, d * 128:(d + 1) * 128],
                            q16g[:, d * FB:(d + 1) * FB],
                            start=(d == 0), stop=(d == NC - 1))
                    nc.vector.tensor_copy(brt[:, :], ps_brt[:, :])

                # ---- spatial rest (accumulates onto the prestart) ----
                brt_v = brt[:, :].rearrange("p (t l) -> p t l", l=L)
                for k in range(NT):
                    for hb in range(2):
                        nc.tensor.matmul(
                            sp_cur[hb][32:53, :], brt_v[:, k, :],
                            kyx_v[:, k, hb * 512:(hb + 1) * 512],
                            start=False, stop=(k == NT - 1),
                            skip_group_check=True)

                # ---- P: reduce gathered partials (or copy local P0) ----
                sbP = wp.tile([NF, L], f16, tag="sbp")
                if it == 0:
                    nc.scalar.copy(sbP[:, :], psP0[:, :])
                else:
                    pv = slots[0:NF, :].rearrange(
                        "p (d f) -> p d f", d=NC)[:, :, FB:FB + 2 * L].bitcast(f16)
                    t1 = wp.tile([NF, 4 * L], f32, tag="pt1")
                    t1v = t1[:, :].rearrange("p (a l) -> p a l", l=L)
                    nc.vector.tensor_tensor(
                        t1v, pv[:, 0:4, :], pv[:, 4:8, :], OP.add)
                    t2 = wp.tile([NF, 2 * L], f32, tag="pt2")
                    t2v = t2[:, :].rearrange("p (a l) -> p a l", l=L)
                    nc.vector.tensor_tensor(t2v, t1v[:, 0:2, :], t1v[:, 2:4, :], OP.add)
                    nc.vector.tensor_tensor(sbP[:, :], t2v[:, 0, :], t2v[:, 1, :], OP.add)

                # ---- bilateral message into rows 0:21 of the same tiles ----
                for hb in range(2):
                    nc.tensor.matmul(
                        sp_cur[hb][0:L, :], sbP[:, :],
                        sb_featc[:, hb * 512:(hb + 1) * 512],
                        start=True, stop=True, skip_group_check=True)

                # ---- cat assembly: 2 wide copies, ACT/DVE on separate tiles
                nc.scalar.copy(cats[0][0:53, :], sp_cur[0][:, :])
                nc.vector.tensor_copy(cats[1][0:53, :], sp_cur[1][:, :])

                # ---- mixing matmul (+unary via identity rows) ----
                psct = ps_ct_p.tile([128, FB], f32, tag="ct")
                for g in range(NT):
                    h, tl = g // 4, g % 4
                    nc.tensor.matmul(
                        psct[:, g * L:(g + 1) * L],
                        cats[h][:, tl * 128:(tl + 1) * 128],
                        sb_w12t[:, :], start=True, stop=True)

                # ---- softmax ----
                ex = wp.tile([128, FB], f32, tag="ex")
                nc.scalar.activation(ex[:, :], psct[:, :], AF.Exp)
                ex_v = ex[:, :].rearrange("p (t l) -> p t l", l=L)
                red = wp.tile([128, NT], f32, tag="red")
                nc.vector.tensor_reduce(red[:, :], ex_v, mybir.AxisListType.X, OP.add)
                rcp = wp.tile([128, NT], f32, tag="rcp")
                nc.vector.reciprocal(rcp[:, :], red[:, :])
                if last:
                    out_v = sb_out[:, :].rearrange("p (t l) -> p t l", l=L)
                    nc.vector.tensor_tensor(
                        out_v, ex_v, rcp[:, :].broadcast_to([128, NT, L]), OP.mult)
                    nc.sync.dma_start(out_d[:, :], sb_out[:, :])
                else:
                    qblk = wp.tile([128, FB], f16, tag="qblk")
                    q_v = qblk[:, :].rearrange("p (t l) -> p t l", l=L)
                    nc.vector.tensor_tensor(
                        q_v, ex_v, rcp[:, :].broadcast_to([128, NT, L]), OP.mult)
                    stag = stp.tile([128, CCW], f8, tag="stag")
                    nc.vector.tensor_copy(stag[:, 0:FB], qblk[:, :])
                    cc_in = dp.tile([128, CCW], f8, tag="ccin")
                    cc_out = dp.tile([128 * NC, CCW], f8, tag="ccout")
                    # ship the q part immediately; P follows as a tiny DMA
                    # launched from the scalar queue (overlapped launches)
                    nc.sync.dma_start(cc_in[:, 0:FB], stag[:, 0:FB])
                    psP = ps_p.tile([NF, L], f32, tag="pp")
                    for t in range(NT):
                        nc.tensor.matmul(
                            psP[:, :], sb_featrt[:, t * NF:(t + 1) * NF],
                            qblk[:, t * L:(t + 1) * L],
                            start=(t == 0), stop=(t == NT - 1))
                    nc.scalar.copy(
                        stag[0:NF, FB:FB + 2 * L].bitcast(f16), psP[:, :])
                    nc.scalar.dma_start(
                        cc_in[0:NF, FB:FB + 2 * L], stag[0:NF, FB:FB + 2 * L])
                    bo = wp.tile([128, FB], f16, tag="bo")
                    nc.vector.tensor_scalar_mul(
                        bo[:, :], qblk[:, :], sb_zcoo[:, 0:1])
                    nc.gpsimd.collective_compute(
                        "AllGather", mybir.AluOpType.bypass,
                        replica_groups=[list(range(NC))],
                        ins=[cc_in.opt()], outs=[cc_out.opt()],
                    )
                    # prestart next iteration's own-slice spatial during CC
                    sp_next = sp_tiles[(it + 1) % 2]
                    bo_v = bo[:, :].rearrange("p (t l) -> p t l", l=L)
                    for k in range(NT):
                        for hb in range(2):
                            nc.tensor.matmul(
                                sp_next[hb][32:53, :],
                                bo_v[:, k, :],
                                kyx_v[:, k, hb * 512:(hb + 1) * 512],
                                start=(k == 0), stop=False,
                                skip_group_check=True)
                    # unload gathered q + P partials: 4 two-rank DMAs whose
                    # launches run on four different sequencers in parallel
                    slots = slp.tile([128, NC * CCW], f8, tag="slots")
                    for d0, dn, eng in ((0, 3, nc.sync), (3, 3, nc.scalar),
                                       (6, 2, nc.gpsimd)):
                        eng.dma_start(
                            slots[:, d0 * CCW:(d0 + dn) * CCW].rearrange(
                                "p (d f) -> p d f", d=dn),
                            cc_out[d0 * 128:(d0 + dn) * 128, :].rearrange(
                                "(d p) f -> p d f", p=128))
                    sp_cur = sp_next
    nc.compile()
    return nc


def _host_prep(image, logits):
    """Per-core input dicts. q0/featrt blocks are permuted own-slice-first."""
    img = np.asarray(image, dtype=np.float32)[0]      # [3, D, H, W]
    lg = np.asarray(logits, dtype=np.float32)[0]      # [L, D, H, W]

    zz, yy, xx = np.meshgrid(
        np.arange(D), np.arange(H), np.arange(W), indexing="ij")
    pos = np.stack([zz, yy, xx], -1).reshape(N, 3).astype(np.float32)
    rgb = img.reshape(3, N).T
    feat = np.concatenate([pos / ALPHA, rgb / BETA], axis=1).astype(np.float16)
    f = feat.astype(np.float32)                       # [N, 6] fp16-rounded
    sq = np.sum(f * f, axis=1)
    al = PA / 2 - sq / 2                              # alpha == beta per formula

    pairs = [(a, b) for a in range(6) for b in range(a + 1, 6)]

    def mono_row(ff, alv):
        cols = [PC * alv * alv, PC * np.ones_like(alv), PC * 2 * alv]
        cols += [PC * 2 * alv * ff[:, a] for a in range(6)]
        cols += [PC * 2 * ff[:, a] for a in range(6)]
        cols += [PC * ff[:, a] ** 2 for a in range(6)]
        cols += [PC * 2 * ff[:, a] * ff[:, b] for a, b in pairs]
        return np.stack(cols, 0)                      # [36, n]

    def mono_col(ff, bev):
        cols = [np.ones_like(bev), bev * bev, bev]
        cols += [ff[:, a] for a in range(6)]
        cols += [bev * ff[:, a] for a in range(6)]
        cols += [ff[:, a] ** 2 for a in range(6)]
        cols += [ff[:, a] * ff[:, b] for a, b in pairs]
        return np.stack(cols, 0)                      # [36, n]

    featr = mono_row(f, al).astype(np.float16)        # [36, N]
    rsum = featr.astype(np.float32).sum(axis=1)       # [36] for the normalizer
    # voxel-major feature chunks: [128, d, t, f]
    featrt_blk = featr.reshape(NF, D, NT, 128).transpose(3, 1, 2, 0)

    r1 = np.arange(D, dtype=np.float32)
    Gz = np.exp(-0.5 * ((r1[:, None] - r1[None, :]) / GAMMA) ** 2)
    r2 = np.arange(H, dtype=np.float32)
    Gy = np.exp(-0.5 * ((r2[:, None] - r2[None, :]) / GAMMA) ** 2)
    Kyx = np.kron(Gy, Gy).astype(np.float32)          # H == W so Gy == Gx
    nyx = Kyx.sum(axis=0)
    Kyx_n = (Kyx / nyx[None, :]).astype(np.float16)
    czsum = Gz.sum(axis=0)
    kyx_in = np.ascontiguousarray(
        Kyx_n.reshape(NT, 128, NYX).transpose(1, 0, 2).reshape(128, NT * NYX))

    unary = lg.reshape(L, N)
    # voxel-major blocks: blkT[p, d, t*L+l] = unary[l, d*NYX + t*128 + p]
    blkT = unary.astype(np.float16).reshape(L, D, NT, 128).transpose(3, 1, 2, 0)

    eye = np.eye(128, dtype=np.float32)
    maps = []
    for r in range(NC):
        sl = slice(r * NYX, (r + 1) * NYX)
        featc = mono_col(f[sl], al[sl]).astype(np.float16)      # [36, 1024]
        norm = rsum @ featc.astype(np.float32)                  # [1024]
        featc_n = (featc.astype(np.float32)
                   * (FSCALE / norm)[None, :]).astype(np.float16)
        perm = [r] + [d for d in range(NC) if d != r]
        un = np.ascontiguousarray(blkT[:, perm].reshape(128, NC * FB))
        featrt = np.ascontiguousarray(
            featrt_blk[:, perm].reshape(128, NC * NT * NF))
        zvec = (Gz[:, r] / czsum[r]).astype(np.float32)
        zcor0 = zvec[perm].copy()
        zcor0[0] = 0.0
        zcor = zvec.copy()
        zcor[r] = 0.0                                 # own slice via prestart
        zid = np.ascontiguousarray(
            (eye[None, :, :] * zcor[:, None, None])
            .transpose(1, 0, 2).reshape(128, NC * 128).astype(np.float16))
        unownt = np.ascontiguousarray(unary[:, sl].astype(np.float16))
        maps.append({
            "unaryt": un,
            "featrt": featrt,
            "featc": np.ascontiguousarray(featc_n),
            "kyx": kyx_in,
            "zid": zid,
            "zcor0": np.ascontiguousarray(np.tile(zcor0, (128, 1))),
            "zcoo": np.full((128, 1), zvec[r], np.float32),
            "unownt": unownt,
        })
    return maps


def _host_w12t(spatial_ker_weights, bilateral_ker_weights, compatibility_matrix):
    ws = np.asarray(spatial_ker_weights, np.float32)
    wb = np.asarray(bilateral_ker_weights, np.float32)
    cm = np.asarray(compatibility_matrix, np.float32)
    w = np.zeros((CATP, L), np.float32)
    w[0:L] = (cm @ wb).T / FSCALE
    w[32:53] = (cm @ ws).T
    w[64:CATP] = np.eye(L)
    return np.ascontiguousarray(w.astype(np.float16))


def kernel(image, logits, spatial_ker_weights, bilateral_ker_weights,
           compatibility_matrix):
    from concourse.bass_utils import run_bass_kernel_spmd

    if "nc" not in _CACHE:
        _CACHE["nc"] = _build_nc()
    nc = _CACHE["nc"]

    maps = _host_prep(image, logits)
    w12t = _host_w12t(spatial_ker_weights, bilateral_ker_weights,
                      compatibility_matrix)
    for m in maps:
        m["w12t"] = w12t

    res = run_bass_kernel_spmd(nc, maps, core_ids=list(range(NC)))

    out = np.empty((L, D, H, W), dtype=np.float32)
    for r in range(NC):
        blk = res.results[r]["out"]                   # [128, 168]
        out[:, r] = blk.reshape(128, NT, L).transpose(2, 1, 0).reshape(L, H, W)
    return out[None]
